# revision 7
# baseline (speedup 1.0000x reference)
"""Trainium2 Bass kernel for the HCFDA dense-CNN module (bf16 pipeline, v2).

Math (exact reassociations of the reference):
  1. The 256x256 1x1 DCT conv is only consumed through a channel-mean, so
     temp[b,h,w] = sum_c m[c] * x[b,c,h,w]  with  m = dct_w.mean(axis=0).
  2. The 3 reflect-pad diffusion steps collapse (host-side) into
     T3 = sum_k M_k @ T @ (Sw^T)^k  -> 3 shift-adds + 4 matmuls on device.
  3. SE branch: pooled stats -> two tiny FCs -> sigmoid (batched both
     branches through one relu / per-t one sigmoid).
  out = x * sigmoid(att[c] * sigmoid(T3)[hw])     (exact or per-channel
  Taylor-linear sigmoid sc ~= A[c] + B[c]*heat, max err ~2e-4)

v2 structural changes vs v1 (94.6us):
  - x is host-interleaved to [128, 2, HW] so ONE DMA loads both channel
    halves of a chunk; 6 input DMAs issued up-front on the Sync queue
    with nothing dependent in front of them (v1 serialized x loads
    behind semaphore-waiting scatter DMAs on the one queue -> 35us
    input phase).  Dependent DMAs (Tp scatter, hrow) go on the Scalar
    HWDGE queue; outputs go on Sync after the x loads.
  - All constants packed into 2 DMAs (one bf16 blob, one f32 blob).
  - GEMV psum staged in 3 wide tiles (2048/1024/1024) -> 3 copies +
    3 scatters instead of 9+9, with a short last-stage tail.
  - Pooled stats: DVE tensor_reduce (bf16 2x) + ACT copy-accum + Pool
    (gpsimd) max-folds, balanced so no engine exceeds the DMA window.
  - Phase B: q0-q4 exact (PE ones-broadcast -> ACT sigmoid(scale=att)
    -> DVE mul), q5 Taylor via PE B-row broadcast + fused DVE stt,
    q6-q7 Taylor via gpsimd partition_broadcast of heat + DVE
    tensor_scalar/mul (no PE, no ACT).  One 2MB output DMA per q
    (t-interleaved), last q split for a short tail.

Sharding: pure data parallel, one batch element per NeuronCore (B=8).
"""

import numpy as np
from contextlib import ExitStack

B, C, H, W = 8, 256, 128, 128
HW = H * W           # 16384
N_CORES = 8
H0 = 0.4975          # heat-range center for the Taylor-linear sigmoid

# input chunks (hw offset, width); small first chunk for an early
# pipeline start, small last chunk for a short stats tail
CHUNKS = ((0, 1024), (1024, 3072), (4096, 4096), (8192, 4096),
          (12288, 3072), (15360, 1024))

# GEMV psum stages: (hw0, hwlen, psum width).  Stage s holds
# hwlen/512 GEMV rows packed k-major: row G_l at tile_position col
# 32*(G_l // (w/512)), psum cols 512*(G_l % (w/512)).
STAGES = ((0, 8192, 2048), (8192, 4096, 1024), (12288, 4096, 1024))

# stats engine assignment: (j, t) -> (sum_engine, max_engine)
SUM_ACT = {(1, 0), (2, 0), (3, 0), (4, 0), (5, 1)}
MAX_POOL = set()

# phase B: per-q mode. 'E' exact (PE pb + ACT sigmoid + DVE mul),
# 'T' taylor via PE B-row broadcast + DVE stt,
# 'P' taylor via gpsimd partition_broadcast + DVE ts/mul.
QMODE = ('E', 'E', 'E', 'E', 'E', 'T', 'P', 'P')
CQ = 2048
NQ = HW // CQ


def _reflect(i, n):
    if i < 0:
        return -i
    if i >= n:
        return 2 * (n - 1) - i
    return i


def _build_program(key_unused):
    from concourse import bass, mybir, tile
    from concourse import bacc

    f32 = mybir.dt.float32
    bf16 = mybir.dt.bfloat16
    AF = mybir.ActivationFunctionType
    ALU = mybir.AluOpType
    AX = mybir.AxisListType

    nc = bacc.Bacc("TRN2", target_bir_lowering=False, debug=False,
                   num_devices=N_CORES)

    xb = nc.dram_tensor("xb", [128, 2, HW], bf16, kind="ExternalInput").ap()
    cb = nc.dram_tensor("cb", [128, 644], bf16, kind="ExternalInput").ap()
    cf = nc.dram_tensor("cf", [128, 548], f32, kind="ExternalInput").ap()
    outd = nc.dram_tensor("out", [128, 2, HW], bf16,
                          kind="ExternalOutput").ap()

    with tile.TileContext(nc) as tc, ExitStack() as ctx:
        const = ctx.enter_context(tc.tile_pool(name="const", bufs=1))
        xpool = ctx.enter_context(tc.tile_pool(name="xp", bufs=1))
        work = ctx.enter_context(tc.tile_pool(name="work", bufs=2))
        stat = ctx.enter_context(tc.tile_pool(name="stat", bufs=1))
        actxA = ExitStack()
        psA = [actxA.enter_context(
            tc.tile_pool(name=f"psA{s}", bufs=1, space="PSUM"))
            for s in range(3)]
        actx = ctx.enter_context(ExitStack())

        # ---- constant + x loads: Sync queue, nothing dependent ahead ----
        cb_sb = const.tile([128, 644], bf16, tag="cb", name="cb")
        nc.sync.dma_start(out=cb_sb[:], in_=cb)
        cf_sb = const.tile([128, 548], f32, tag="cf", name="cf")
        nc.sync.dma_start(out=cf_sb[:], in_=cf)
        xt = {}
        for j, (joff, jsz) in enumerate(CHUNKS):
            xt[j] = xpool.tile([128, 2 * jsz], bf16, tag=f"x{j}",
                               name=f"x{j}")
            nc.sync.dma_start(out=xt[j][:], in_=xb[:, :, joff:joff + jsz])

        # const blob views
        m_sb = cb_sb[:, 0:2]                       # [128,2] GEMV vector
        mt_sb = [cb_sb[:, 2 + 128 * k:2 + 128 * (k + 1)] for k in range(4)]
        on_sb = cb_sb[0:1, 514:642]                # [1,128] ones row
        w1_sb = cf_sb[:, 0:32]
        b2c_sb = cf_sb[:, 32:34]
        w2_sb = cf_sb[0:16, 34:290]
        b1_sb = cf_sb[0:16, 290:291]
        b2r2_sb = cf_sb[0:1, 291:547]

        warm = const.tile([1, 2], f32, tag="warm", name="warm")
        nc.scalar.activation(warm[:], cf_sb[0:1, 0:2], AF.Sigmoid)

        sums = stat.tile([128, 2, 6], f32, tag="sums", name="sums")
        rmax = stat.tile([128, 2, 6], f32, tag="rmax", name="rmax")
        junkA = stat.tile([128, 4096], bf16, tag="junkA", name="junkA")
        Tp = [stat.tile([128, W + 2], bf16, tag=f"Tp{i}", name=f"Tp{i}")
              for i in range(4)]
        heat = stat.tile([128, W], bf16, tag="heat", name="heat")
        hrow = stat.tile([1, HW], bf16, tag="hrow", name="hrow")
        stg = [stat.tile([128, STAGES[s][2]], bf16, tag=f"stg{s}",
                         name=f"stg{s}") for s in range(3)]
        pfold = {}

        # psum stage tiles
        pss = [psA[s].tile([128, STAGES[s][2]], f32, tag=f"ps{s}",
                           name=f"ps{s}") for s in range(3)]

        def stage_of(hw0):
            for s, (s0, slen, sw) in enumerate(STAGES):
                if s0 <= hw0 < s0 + slen:
                    return s, s0, sw
            raise AssertionError(hw0)

        def emit_gemv(j):
            joff, jsz = CHUNKS[j]
            for G0 in range(joff, joff + jsz, 512):
                s, s0, sw = stage_of(G0)
                gl = (G0 - s0) // 512
                ncol = sw // 512
                k, g = gl // ncol, gl % ncol
                dst = pss[s][32 * k:32 * k + 1, 512 * g:512 * g + 512]
                co = G0 - joff
                for t in range(2):
                    nc.tensor.matmul(
                        dst, m_sb[:, t:t + 1],
                        xt[j][:, t * jsz + co:t * jsz + co + 512],
                        start=(t == 0), stop=(t == 1),
                        tile_position=(0, 32 * k))
            # stage completed by this chunk?  copy + scatter
            for s, (s0, slen, sw) in enumerate(STAGES):
                if s0 + slen == joff + jsz:
                    nc.scalar.activation(stg[s][:], pss[s][:], AF.Copy)
                    r0 = s0 // 128
                    nr = slen // 128
                    nc.scalar.dma_start(
                        out=Tp[0][r0:r0 + nr, 1:W + 1],
                        in_=stg[s][0:128:32, :])

        def emit_stats(j):
            joff, jsz = CHUNKS[j]
            for t in range(2):
                xs = xt[j][:, t * jsz:(t + 1) * jsz]
                if (j, t) in SUM_ACT:
                    nc.scalar.activation(
                        junkA[:, 0:jsz], xs, AF.Copy,
                        accum_out=sums[:, t:t + 1, j:j + 1])
                else:
                    nc.vector.tensor_reduce(
                        sums[:, t:t + 1, j:j + 1], xs, axis=AX.X,
                        op=ALU.add)
                if (j, t) in MAX_POOL:
                    h1 = jsz // 2
                    f1 = stat.tile([128, h1], bf16, tag=f"pf{j}_{t}",
                                   name=f"pf{j}_{t}")
                    nc.gpsimd.tensor_tensor(f1[:], xs[:, 0:h1],
                                            xs[:, h1:jsz], op=ALU.max)
                    pfold[(j, t)] = f1
                else:
                    nc.vector.tensor_reduce(
                        rmax[:, t:t + 1, j:j + 1], xs, axis=AX.X,
                        op=ALU.max)

        for j in range(len(CHUNKS)):
            with tc.high_priority():
                emit_gemv(j)
            emit_stats(j)

        # finish the gpsimd max folds on DVE
        for (j, t), f1 in pfold.items():
            nc.vector.tensor_reduce(rmax[:, t:t + 1, j:j + 1], f1[:],
                                    axis=AX.X, op=ALU.max)

        # ---- fused diffusion on Tp + heat ----
        actxA.close()  # free GEMV psum banks
        psD = actx.enter_context(tc.tile_pool(name="psD", bufs=1,
                                              space="PSUM"))
        psF = actx.enter_context(tc.tile_pool(name="psF", bufs=1,
                                              space="PSUM"))
        with tc.high_priority():
            nc.vector.tensor_copy(Tp[0][:, 0:1], Tp[0][:, 2:3])
            nc.vector.tensor_copy(Tp[0][:, W + 1:W + 2], Tp[0][:, W - 1:W])
            pd3 = psD.tile([128, W], f32, tag="psD", name="psD")
            for k in range(4):
                nc.tensor.matmul(pd3[:], mt_sb[k], Tp[k][:, 1:W + 1],
                                 start=(k == 0), stop=(k == 3))
                if k < 3:
                    nxt = Tp[k + 1]
                    nc.vector.tensor_add(nxt[:, 1:W + 1], Tp[k][:, 0:W],
                                         Tp[k][:, 2:W + 2])
                    nc.vector.tensor_copy(nxt[:, 0:1], nxt[:, 2:3])
                    nc.vector.tensor_copy(nxt[:, W + 1:W + 2],
                                          nxt[:, W - 1:W])
            nc.scalar.activation(heat[:], pd3[:], AF.Sigmoid)
            nc.scalar.dma_start(out=hrow[0:1, 0:HW // 2], in_=heat[0:64, :])
            nc.scalar.dma_start(out=hrow[0:1, HW // 2:HW],
                                in_=heat[64:128, :])

        # ---- stats finalize + batched SE FC chain ----
        ysum = stat.tile([128, 2], f32, tag="ysum", name="ysum")
        ymax = stat.tile([128, 2], f32, tag="ymax", name="ymax")
        att = stat.tile([128, 2], f32, tag="att", name="att")
        Yb = stat.tile([128, 4], f32, tag="Yb", name="Yb")
        for t in range(2):
            nc.vector.tensor_reduce(ysum[:, t:t + 1], sums[:, t:t + 1, :],
                                    axis=AX.XY, op=ALU.add)
            nc.vector.tensor_reduce(ymax[:, t:t + 1], rmax[:, t:t + 1, :],
                                    axis=AX.XY, op=ALU.max)
            nc.vector.tensor_scalar_mul(Yb[:, 2 * t:2 * t + 1],
                                        ysum[:, t:t + 1], 1.0 / HW)
            nc.vector.tensor_copy(Yb[:, 2 * t + 1:2 * t + 2],
                                  ymax[:, t:t + 1])
        ph = psF.tile([16, 2], f32, tag="psF", name="ph")
        nc.tensor.matmul(ph[:], w1_sb[:, 0:16], Yb[:, 0:2],
                         start=True, stop=False)
        nc.tensor.matmul(ph[:], w1_sb[:, 16:32], Yb[:, 2:4],
                         start=False, stop=True)
        hb = stat.tile([16, 2], f32, tag="hb", name="hb")
        nc.scalar.activation(hb[:], ph[:], AF.Relu, bias=b1_sb)
        attr = {}
        for t in range(2):
            pa = psF.tile([128, 2], f32, tag="psFa", name=f"pa{t}")
            nc.tensor.matmul(pa[:], w2_sb[:, 128 * t:128 * (t + 1)], hb[:],
                             start=True, stop=True)
            sg = stat.tile([128, 2], f32, tag=f"sg{t}", name=f"sg{t}")
            nc.scalar.activation(sg[:], pa[:], AF.Sigmoid,
                                 bias=b2c_sb[:, t:t + 1])
            nc.vector.tensor_add(att[:, t:t + 1], sg[:, 0:1], sg[:, 1:2])
            if 'T' in QMODE:
                srs = []
                for br in range(2):
                    par = psF.tile([1, 128], f32, tag="psFr",
                                   name=f"par{t}_{br}")
                    nc.tensor.matmul(par[:], hb[:, br:br + 1],
                                     w2_sb[:, 128 * t:128 * (t + 1)],
                                     start=True, stop=True)
                    sr = stat.tile([1, 128], f32, tag=f"sr{t}_{br}",
                                   name=f"sr{t}_{br}")
                    nc.vector.tensor_add(sr[:], par[:],
                                         b2r2_sb[:, 128 * t:128 * (t + 1)])
                    nc.scalar.activation(sr[:], sr[:], AF.Sigmoid)
                    srs.append(sr)
                ar = stat.tile([1, 128], f32, tag=f"ar{t}", name=f"ar{t}")
                nc.vector.tensor_add(ar[:], srs[0][:], srs[1][:])
                attr[t] = ar

        # Taylor-linear sigmoid coeffs around u = att*H0:
        #   sc ~= A + Bc*heat,  A = s - u*s' (col),  Bc = att*s' (col)
        uat = stat.tile([128, 2], f32, tag="uat", name="uat")
        nc.vector.tensor_scalar_mul(uat[:], att[:], H0)
        sat = stat.tile([128, 2], f32, tag="sat", name="sat")
        nc.scalar.activation(sat[:], uat[:], AF.Sigmoid)
        spt = stat.tile([128, 2], f32, tag="spt", name="spt")
        nc.vector.tensor_mul(spt[:], sat[:], sat[:])
        nc.vector.tensor_sub(spt[:], sat[:], spt[:])       # s*(1-s)
        Abf = stat.tile([128, 2], f32, tag="Abf", name="Abf")
        nc.vector.tensor_mul(Abf[:], uat[:], spt[:])
        nc.vector.tensor_sub(Abf[:], sat[:], Abf[:])
        Bcol = stat.tile([128, 2], f32, tag="Bcol", name="Bcol")
        nc.vector.tensor_mul(Bcol[:], att[:], spt[:])
        Brow = {}
        for t in attr:
            uar = stat.tile([1, 128], f32, tag=f"uar{t}", name=f"uar{t}")
            nc.vector.tensor_scalar_mul(uar[:], attr[t][:], H0)
            sar = stat.tile([1, 128], f32, tag=f"sar{t}", name=f"sar{t}")
            nc.scalar.activation(sar[:], uar[:], AF.Sigmoid)
            spr = stat.tile([1, 128], f32, tag=f"spr{t}", name=f"spr{t}")
            nc.vector.tensor_mul(spr[:], sar[:], sar[:])
            nc.vector.tensor_sub(spr[:], sar[:], spr[:])
            br = stat.tile([1, 128], bf16, tag=f"Brow{t}", name=f"Brow{t}")
            nc.vector.tensor_mul(br[:], attr[t][:], spr[:])
            Brow[t] = br

        # ---- Phase B: out = x * attention ----
        actx.close()   # free phase-A/mid PSUM banks

        def xpieces(t, hw0, width):
            out = []
            pos = hw0
            while pos < hw0 + width:
                for jj, (joff, jsz) in enumerate(CHUNKS):
                    if joff <= pos < joff + jsz:
                        w_ = min(hw0 + width, joff + jsz) - pos
                        out.append((pos - hw0,
                                    xt[jj][:, t * jsz + pos - joff:
                                           t * jsz + pos - joff + w_],
                                    w_))
                        pos += w_
                        break
                else:
                    raise AssertionError(pos)
            return out

        with tc.tile_pool(name="psB", bufs=2, space="PSUM") as psB:
            for q in range(NQ):
                mode = QMODE[q]
                o = work.tile([128, 2 * CQ], bf16, tag="o", name=f"o{q}",
                              bufs=3)
                if mode == 'E':
                    pb = psB.tile([128, CQ], f32, tag="psB", name=f"pb{q}")
                    for ss in range(CQ // 512):
                        c0 = q * CQ + ss * 512
                        nc.tensor.matmul(pb[:, ss * 512:(ss + 1) * 512],
                                         on_sb, hrow[0:1, c0:c0 + 512],
                                         start=True, stop=True)
                    for t in range(2):
                        sc = work.tile([128, CQ], bf16, tag="sc",
                                       name=f"sc{q}_{t}", bufs=3)
                        nc.scalar.activation(sc[:], pb[:], AF.Sigmoid,
                                             scale=att[:, t:t + 1])
                        for (rel, xap, w_) in xpieces(t, q * CQ, CQ):
                            nc.vector.tensor_mul(
                                o[:, t * CQ + rel:t * CQ + rel + w_],
                                xap, sc[:, rel:rel + w_])
                elif mode == 'T':
                    for t in range(2):
                        pbt = psB.tile([128, CQ], f32, tag="psB",
                                       name=f"pbt{q}_{t}")
                        for ss in range(CQ // 512):
                            c0 = q * CQ + ss * 512
                            nc.tensor.matmul(
                                pbt[:, ss * 512:(ss + 1) * 512],
                                Brow[t][:], hrow[0:1, c0:c0 + 512],
                                start=True, stop=True)
                        for (rel, xap, w_) in xpieces(t, q * CQ, CQ):
                            nc.vector.scalar_tensor_tensor(
                                o[:, t * CQ + rel:t * CQ + rel + w_],
                                pbt[:, rel:rel + w_], Abf[:, t:t + 1],
                                xap, op0=ALU.add, op1=ALU.mult)
                else:  # 'P': gpsimd heat broadcast + DVE ts/mul
                    hbb = work.tile([128, CQ], bf16, tag="hbb",
                                    name=f"hbb{q}", bufs=2)
                    nc.gpsimd.partition_broadcast(
                        hbb[:], hrow[0:1, q * CQ:(q + 1) * CQ])
                    for t in range(2):
                        sc = work.tile([128, CQ], bf16, tag="sc",
                                       name=f"sc{q}_{t}", bufs=3)
                        nc.vector.tensor_scalar(
                            sc[:], hbb[:], Bcol[:, t:t + 1],
                            Abf[:, t:t + 1], op0=ALU.mult, op1=ALU.add)
                        for (rel, xap, w_) in xpieces(t, q * CQ, CQ):
                            nc.vector.tensor_mul(
                                o[:, t * CQ + rel:t * CQ + rel + w_],
                                xap, sc[:, rel:rel + w_])
                if q == NQ - 1:
                    for hh in range(2):
                        nc.sync.dma_start(
                            out=outd[:, hh:hh + 1,
                                     q * CQ:(q + 1) * CQ],
                            in_=o[:, hh * CQ:(hh + 1) * CQ])
                else:
                    nc.sync.dma_start(
                        out=outd[:, :, q * CQ:(q + 1) * CQ],
                        in_=o[:])

    nc.compile()
    return nc


_prog_cache = {}
_TRACE = False      # test harness sets True to collect an NTFF profile
_last_res = None    # BassKernelResults of the most recent run


def kernel(x, dct_w, w1, b1, w2, b2, alpha, lap):
    import ml_dtypes

    x = np.asarray(x, dtype=np.float32)
    dct_w = np.asarray(dct_w, dtype=np.float32)
    w1 = np.asarray(w1, dtype=np.float32)
    b1 = np.asarray(b1, dtype=np.float32)
    w2 = np.asarray(w2, dtype=np.float32)
    b2 = np.asarray(b2, dtype=np.float32)
    alpha = float(np.asarray(alpha))
    lap = np.asarray(lap, dtype=np.float64)

    # decomposition requires the kernel's row structure (holds for HCFDA's
    # fixed Laplacian); verify.
    assert np.allclose(lap[0], lap[2]) and np.allclose(lap[:, 0], lap[:, 2])
    a, b = float(lap[0, 0]), float(lap[0, 1])

    m = dct_w.astype(np.float64).mean(axis=0)           # [C]
    S = np.zeros((H, H), dtype=np.float64)
    for h in range(H):
        S[h, _reflect(h - 1, H)] += 1.0
        S[h, _reflect(h + 1, H)] += 1.0
    # fused 3-step diffusion: T3 = sum_k C(3,k) P^(3-k) Q^k @ T @ (Sw^T)^k
    from math import comb
    G = (alpha * a) * S
    c24 = 1.0 + alpha * float(lap[1, 1])
    P = c24 * np.eye(H) + 4.0 * G
    Q = (alpha * b) * np.eye(H) + G
    mts = [np.linalg.matrix_power(P, 3 - k) @ np.linalg.matrix_power(Q, k)
           * comb(3, k) for k in range(4)]

    bf16 = ml_dtypes.bfloat16

    # bf16 const blob [128, 644]: mv(2) | mt0..3 (4*128) | ones row (128)
    cbh = np.zeros((128, 644), dtype=np.float32)
    cbh[:, 0:2] = np.ascontiguousarray(
        m.astype(np.float32).reshape(2, 128).T)
    for k in range(4):
        cbh[:, 2 + 128 * k:2 + 128 * (k + 1)] = mts[k].T
    cbh[0, 514:642] = 1.0
    cbh = cbh.astype(bf16)

    # f32 const blob [128, 420]:
    # w1t(32) | b2c(2) | w2t rows0:16 (256) | b1 col (1) | b2 row (256)
    cfh = np.zeros((128, 548), dtype=np.float32)
    cfh[:, 0:32] = w1.T.reshape(2, 128, 16).transpose(1, 0, 2).reshape(
        128, 32)
    cfh[:, 32:34] = b2.reshape(2, 128).T
    cfh[0:16, 34:290] = w2.T
    cfh[0:16, 290] = b1
    cfh[0, 291:547] = b2

    key = 0
    if key not in _prog_cache:
        _prog_cache[key] = _build_program(key)
    nc = _prog_cache[key]

    # host-interleave x: [B, C, HW] -> per core [128, 2, HW]
    xr = x.reshape(B, 2, 128, HW).transpose(0, 2, 1, 3)
    xr = np.ascontiguousarray(xr).astype(bf16)
    consts = {"cb": cbh, "cf": cfh}
    in_maps = [{"xb": xr[i], **consts} for i in range(N_CORES)]

    from concourse.bass_utils import run_bass_kernel_spmd
    res = run_bass_kernel_spmd(nc, in_maps, list(range(N_CORES)),
                               trace=_TRACE)
    global _last_res
    _last_res = res
    out = np.empty((N_CORES, C, H, W), dtype=np.float32)
    for i in range(N_CORES):
        oi = res.results[i]["out"].astype(np.float32)   # [128, 2, HW]
        out[i] = oi.transpose(1, 0, 2).reshape(C, H, W)
    return out


# revision 10
# speedup vs baseline: 1.3640x; 1.3640x over previous
"""Trainium2 Bass kernel for the HCFDA dense-CNN module (bf16 pipeline, v3).

Math (exact reassociations of the reference):
  1. The 256x256 1x1 DCT conv is only consumed through a channel-mean, so
     temp[b,h,w] = sum_c m[c] * x[b,c,h,w]  with  m = dct_w.mean(axis=0).
  2. The 3 reflect-pad diffusion steps collapse (host-side) into
     T3 = sum_k M_k @ T @ (Sw^T)^k  -> 3 shift-adds + 4 matmuls on device.
  3. SE branch: pooled stats -> two tiny FCs -> sigmoid (branches batched,
     weights in bf16).
  out = x * sigmoid(att[c] * heat[hw]),  heat = sigmoid(T3); exact via ACT
  sigmoid(scale=att) or per-channel Taylor sc ~= A[c] + B[c]*heat (max err
  ~2e-4 at the observed heat range).

Measured op rates (HW bench) that drive the design:
  DVE tensor_tensor bf16: 2x (0.55 ns/col), strided 3D views keep 2x ->
    one op covers both channel halves.  tensor_scalar (AP scalars): 4x
    (0.33 ns/col).  tensor_reduce / accum variants / STT: only 1x.
  ACT: 0.9 ns/col any dtype.  gpsimd partition_broadcast [128,2048]:
    ~3.3 us.  PE matmul: ~0.7 ns/col + ~220 ns.
Hence: pooled max/sum via 2x pairwise fold trees into per-chunk partials
(DVE) + ACT copy-accum for a balanced share of the sums; phase B splits
16 output tiles into 5 exact-q (PE heat-broadcast -> ACT sigmoid -> one
wide DVE mul) + 3 taylor-q (Pool heat-broadcast -> 4x DVE tensor_scalar
-> wide DVE mul), hitting the ~23 us HBM write floor with DVE ~20 us.

DMA layout: x host-interleaved to [128, 2, HW]; 6 chunk loads + 2 packed
const loads issued up-front on the Sync queue (nothing dependent ahead);
Tp scatters + hrow ride the Scalar HWDGE queue; outputs on Sync.

Sharding: pure data parallel, one batch element per NeuronCore (B=8).
"""

import numpy as np
from contextlib import ExitStack

B, C, H, W = 8, 256, 128, 128
HW = H * W           # 16384
N_CORES = 8
H0 = 0.4975          # heat-range center for the Taylor-linear sigmoid

CHUNKS = ((0, 1024), (1024, 3072), (4096, 4096), (8192, 4096),
          (12288, 3072), (15360, 1024))

# GEMV psum stages: (hw0, hwlen, psum width)
STAGES = ((0, 8192, 2048), (8192, 4096, 1024), (12288, 4096, 1024))

# sums computed on ACT (copy+accum into `sums` col): (j, t) pairs
SUM_ACT = {(1, 0), (1, 1), (2, 0), (3, 0), (5, 0), (5, 1)}

# phase B: 'E' exact (PE pb + ACT sigmoid + DVE mul), 'P' taylor via
# gpsimd partition_broadcast + 4x DVE tensor_scalar + DVE mul
QMODE = ('E', 'E', 'E', 'E', 'E', 'P', 'P', 'P')
CQ = 2048
NQ = HW // CQ


def _reflect(i, n):
    if i < 0:
        return -i
    if i >= n:
        return 2 * (n - 1) - i
    return i


def _build_program(key_unused):
    from concourse import bass, mybir, tile
    from concourse import bacc

    f32 = mybir.dt.float32
    bf16 = mybir.dt.bfloat16
    AF = mybir.ActivationFunctionType
    ALU = mybir.AluOpType
    AX = mybir.AxisListType

    nc = bacc.Bacc("TRN2", target_bir_lowering=False, debug=False,
                   num_devices=N_CORES)

    xb = nc.dram_tensor("xb", [128, 2, HW], bf16, kind="ExternalInput").ap()
    cb = nc.dram_tensor("cb", [128, 932], bf16, kind="ExternalInput").ap()
    cf = nc.dram_tensor("cf", [128, 4], f32, kind="ExternalInput").ap()
    outd = nc.dram_tensor("out", [128, 2, HW], bf16,
                          kind="ExternalOutput").ap()

    def tview(ap_2d, jsz):
        # [128, 2*jsz] chunk tile -> [128, 2, jsz]
        return ap_2d.rearrange("p (t c) -> p t c", t=2)

    with tile.TileContext(nc) as tc, ExitStack() as ctx:
        const = ctx.enter_context(tc.tile_pool(name="const", bufs=1))
        xpool = ctx.enter_context(tc.tile_pool(name="xp", bufs=1))
        work = ctx.enter_context(tc.tile_pool(name="work", bufs=2))
        stat = ctx.enter_context(tc.tile_pool(name="stat", bufs=1))
        actxA = ExitStack()
        psA = [actxA.enter_context(
            tc.tile_pool(name=f"psA{s}", bufs=1, space="PSUM"))
            for s in range(3)]
        actx = ctx.enter_context(ExitStack())

        # ---- loads: first x chunk, consts, rest of x — all on Sync ----
        xt = {}

        def load_chunk(j):
            joff, jsz = CHUNKS[j]
            xt[j] = xpool.tile([128, 2 * jsz], bf16, tag=f"x{j}",
                               name=f"x{j}")
            nc.sync.dma_start(out=xt[j][:], in_=xb[:, :, joff:joff + jsz])

        load_chunk(0)
        cb_sb = const.tile([128, 932], bf16, tag="cb", name="cb")
        nc.sync.dma_start(out=cb_sb[:], in_=cb)
        cf_sb = const.tile([128, 4], f32, tag="cf", name="cf")
        nc.sync.dma_start(out=cf_sb[:], in_=cf)
        for j in range(1, len(CHUNKS)):
            load_chunk(j)

        # const blob views (bf16): mv 2 | mt 4*128 | ones 128 (row0)
        #   | w1t 32 | w2t 256 (rows 0:16)
        m_sb = cb_sb[:, 0:2]
        mt_sb = [cb_sb[:, 2 + 128 * k:2 + 128 * (k + 1)] for k in range(4)]
        on_sb = cb_sb[0:1, 514:642]
        w1_sb = cb_sb[:, 642:674]
        w2_sb = cb_sb[0:16, 674:930]
        # f32 blob: b2c [128,2] | b1 [16,1] col2 | (col3 spare)
        b2c_sb = cf_sb[:, 0:2]
        b1_sb = cf_sb[0:16, 2:3]

        warm = const.tile([1, 2], f32, tag="warm", name="warm")
        nc.scalar.activation(warm[:], cf_sb[0:1, 0:2], AF.Sigmoid)

        sums = stat.tile([128, 2, 8], f32, tag="sums", name="sums")
        junkA = stat.tile([128, 4096], bf16, tag="junkA", name="junkA")
        Tp = [stat.tile([128, W + 2], bf16, tag=f"Tp{i}", name=f"Tp{i}")
              for i in range(4)]
        heat = stat.tile([128, W], bf16, tag="heat", name="heat")
        hrow = stat.tile([1, HW], bf16, tag="hrow", name="hrow")
        stg = [stat.tile([128, STAGES[s][2]], bf16, tag=f"stg{s}",
                         name=f"stg{s}") for s in range(3)]
        pss = [psA[s].tile([128, STAGES[s][2]], f32, tag=f"ps{s}",
                           name=f"ps{s}") for s in range(3)]

        def stage_of(hw0):
            for s, (s0, slen, sw) in enumerate(STAGES):
                if s0 <= hw0 < s0 + slen:
                    return s, s0, sw
            raise AssertionError(hw0)

        def emit_gemv(j):
            joff, jsz = CHUNKS[j]
            for G0 in range(joff, joff + jsz, 512):
                s, s0, sw = stage_of(G0)
                gl = (G0 - s0) // 512
                ncol = sw // 512
                k, g = gl // ncol, gl % ncol
                dst = pss[s][32 * k:32 * k + 1, 512 * g:512 * g + 512]
                co = G0 - joff
                for t in range(2):
                    nc.tensor.matmul(
                        dst, m_sb[:, t:t + 1],
                        xt[j][:, t * jsz + co:t * jsz + co + 512],
                        start=(t == 0), stop=(t == 1),
                        tile_position=(0, 32 * k))
            for s, (s0, slen, sw) in enumerate(STAGES):
                if s0 + slen == joff + jsz:
                    nc.scalar.activation(stg[s][:], pss[s][:], AF.Copy)
                    r0 = s0 // 128
                    nr = slen // 128
                    nc.scalar.dma_start(
                        out=Tp[0][r0:r0 + nr, 1:W + 1],
                        in_=stg[s][0:128:32, :])

        # per-chunk stat partials (all [128, 2, 1024]-shaped, bf16)
        mpart = {}     # j -> max partial (j0 uses the chunk tile itself)
        spart = {}     # key -> sum partial

        def emit_stats(j):
            joff, jsz = CHUNKS[j]
            x3 = tview(xt[j][:], jsz)
            # ---- ACT sums ----
            for t in range(2):
                if (j, t) in SUM_ACT:
                    nc.scalar.activation(
                        junkA[:, 0:jsz], xt[j][:, t * jsz:(t + 1) * jsz],
                        AF.Copy, accum_out=sums[:, t:t + 1, j:j + 1])
            # ---- DVE max fold to [2,1024] partial ----
            if j == 5:
                return      # tail chunk handled in finalize
            if jsz == 1024:
                mpart[j] = x3
            else:
                mp = stat.tile([128, 2, 1024], bf16, tag=f"mp{j}",
                               name=f"mp{j}")
                if jsz == 4096:
                    f1 = work.tile([128, 2, 2048], bf16, tag="f2k",
                                   name=f"f2k_{j}", bufs=1)
                    nc.vector.tensor_tensor(f1[:], x3[:, :, 0:2048],
                                            x3[:, :, 2048:4096],
                                            op=ALU.max)
                    nc.vector.tensor_tensor(mp[:], f1[:, :, 0:1024],
                                            f1[:, :, 1024:2048],
                                            op=ALU.max)
                else:   # 3072
                    nc.vector.tensor_tensor(mp[:], x3[:, :, 0:1024],
                                            x3[:, :, 1024:2048],
                                            op=ALU.max)
                    nc.vector.tensor_tensor(mp[:], mp[:],
                                            x3[:, :, 2048:3072],
                                            op=ALU.max)
                mpart[j] = mp[:]
            # ---- DVE sum partials for the non-ACT share ----
            if j in (2, 3):     # t=1 only
                x1 = x3[:, 1:2, :]
                sp = stat.tile([128, 1, 1024], bf16, tag=f"sp{j}",
                               name=f"sp{j}")
                f1 = work.tile([128, 1, 2048], bf16, tag="s2k",
                               name=f"s2k_{j}", bufs=1)
                nc.vector.tensor_tensor(f1[:], x1[:, :, 0:2048],
                                        x1[:, :, 2048:4096], op=ALU.add)
                nc.vector.tensor_tensor(sp[:], f1[:, :, 0:1024],
                                        f1[:, :, 1024:2048], op=ALU.add)
                spart[j] = sp
            elif j == 4:        # both t
                sp = stat.tile([128, 2, 1024], bf16, tag="sp4", name="sp4")
                nc.vector.tensor_tensor(sp[:], x3[:, :, 0:1024],
                                        x3[:, :, 1024:2048], op=ALU.add)
                nc.vector.tensor_tensor(sp[:], sp[:], x3[:, :, 2048:3072],
                                        op=ALU.add)
                spart[4] = sp

        for j in range(len(CHUNKS)):
            with tc.high_priority():
                emit_gemv(j)
            emit_stats(j)

        # ---- merge partials (overlaps the j5 transfer) ----
        x0 = tview(xt[0][:], 1024)
        x5 = tview(xt[5][:], 1024)
        n1 = stat.tile([128, 2, 1024], bf16, tag="n1", name="n1")
        nc.vector.tensor_tensor(n1[:], mpart[1], mpart[2][:, :, :],
                                op=ALU.max)
        n2 = stat.tile([128, 2, 1024], bf16, tag="n2", name="n2")
        nc.vector.tensor_tensor(n2[:], mpart[3], mpart[4], op=ALU.max)
        nc.vector.tensor_tensor(n1[:], n1[:], n2[:], op=ALU.max)
        nc.vector.tensor_tensor(n1[:], n1[:], x0, op=ALU.max)
        m1 = stat.tile([128, 2], f32, tag="m1", name="m1")
        nc.vector.tensor_reduce(m1[:], n1[:], axis=AX.X, op=ALU.max)
        # sum merge: Q[t] = P_j0 + P_j4 (+ P_j2t1 + P_j3t1 for t=1)
        qs = stat.tile([128, 2, 1024], bf16, tag="qs", name="qs")
        nc.vector.tensor_tensor(qs[:], x0, spart[4][:], op=ALU.add)
        nc.vector.tensor_tensor(qs[:, 1:2, :], qs[:, 1:2, :],
                                spart[2][:], op=ALU.add)
        nc.vector.tensor_tensor(qs[:, 1:2, :], qs[:, 1:2, :],
                                spart[3][:], op=ALU.add)
        for t in range(2):
            nc.scalar.activation(junkA[:, 0:1024],
                                 qs[:, t, :], AF.Copy,
                                 accum_out=sums[:, t:t + 1, 6:7])
        # ---- j5 tail: per-t max reduce on DVE; sums were on ACT ----
        ymax = stat.tile([128, 2], f32, tag="ymax", name="ymax")
        ysum = stat.tile([128, 2], f32, tag="ysum", name="ysum")
        for t in range(2):
            j5m = stat.tile([128, 1], f32, tag=f"j5m{t}", name=f"j5m{t}")
            nc.vector.tensor_reduce(j5m[:], x5[:, t:t + 1, :], axis=AX.XY,
                                    op=ALU.max)
            nc.vector.tensor_tensor(ymax[:, t:t + 1], m1[:, t:t + 1],
                                    j5m[:], op=ALU.max)
            nc.gpsimd.memset(sums[:, t, 7:8], 0.0)
            nc.vector.tensor_reduce(ysum[:, t:t + 1], sums[:, t:t + 1, :],
                                    axis=AX.XY, op=ALU.add)

        # ---- diffusion on Tp + heat (scalar-queue DMAs) ----
        actxA.close()
        psD = actx.enter_context(tc.tile_pool(name="psD", bufs=1,
                                              space="PSUM"))
        psF = actx.enter_context(tc.tile_pool(name="psF", bufs=1,
                                              space="PSUM"))
        with tc.high_priority():
            nc.vector.tensor_copy(Tp[0][:, 0:1], Tp[0][:, 2:3])
            nc.vector.tensor_copy(Tp[0][:, W + 1:W + 2], Tp[0][:, W - 1:W])
            pd3 = psD.tile([128, W], f32, tag="psD", name="psD")
            for k in range(4):
                nc.tensor.matmul(pd3[:], mt_sb[k], Tp[k][:, 1:W + 1],
                                 start=(k == 0), stop=(k == 3))
                if k < 3:
                    nxt = Tp[k + 1]
                    nc.vector.tensor_add(nxt[:, 1:W + 1], Tp[k][:, 0:W],
                                         Tp[k][:, 2:W + 2])
                    nc.vector.tensor_copy(nxt[:, 0:1], nxt[:, 2:3])
                    nc.vector.tensor_copy(nxt[:, W + 1:W + 2],
                                          nxt[:, W - 1:W])
            nc.scalar.activation(heat[:], pd3[:], AF.Sigmoid)
            nc.scalar.dma_start(out=hrow[0:1, 0:HW // 2], in_=heat[0:64, :])
            nc.scalar.dma_start(out=hrow[0:1, HW // 2:HW],
                                in_=heat[64:128, :])

        # ---- batched SE FC chain (bf16 weights) ----
        att = stat.tile([128, 2], f32, tag="att", name="att")
        Yb = stat.tile([128, 4], bf16, tag="Yb", name="Yb")
        for t in range(2):
            nc.vector.tensor_scalar_mul(Yb[:, 2 * t:2 * t + 1],
                                        ysum[:, t:t + 1], 1.0 / HW)
            nc.vector.tensor_copy(Yb[:, 2 * t + 1:2 * t + 2],
                                  ymax[:, t:t + 1])
        ph = psF.tile([16, 2], f32, tag="psF", name="ph")
        nc.tensor.matmul(ph[:], w1_sb[:, 0:16], Yb[:, 0:2],
                         start=True, stop=False)
        nc.tensor.matmul(ph[:], w1_sb[:, 16:32], Yb[:, 2:4],
                         start=False, stop=True)
        hb = stat.tile([16, 2], bf16, tag="hb", name="hb")
        nc.scalar.activation(hb[:], ph[:], AF.Relu, bias=b1_sb)
        for t in range(2):
            pa = psF.tile([128, 2], f32, tag="psFa", name=f"pa{t}")
            nc.tensor.matmul(pa[:], w2_sb[:, 128 * t:128 * (t + 1)], hb[:],
                             start=True, stop=True)
            sg = stat.tile([128, 2], f32, tag=f"sg{t}", name=f"sg{t}")
            nc.scalar.activation(sg[:], pa[:], AF.Sigmoid,
                                 bias=b2c_sb[:, t:t + 1])
            nc.vector.tensor_add(att[:, t:t + 1], sg[:, 0:1], sg[:, 1:2])

        # Taylor coeffs (column form only): sc ~= A + B*heat
        uat = stat.tile([128, 2], f32, tag="uat", name="uat")
        nc.vector.tensor_scalar_mul(uat[:], att[:], H0)
        sat = stat.tile([128, 2], f32, tag="sat", name="sat")
        nc.scalar.activation(sat[:], uat[:], AF.Sigmoid)
        spt = stat.tile([128, 2], f32, tag="spt", name="spt")
        nc.vector.tensor_mul(spt[:], sat[:], sat[:])
        nc.vector.tensor_sub(spt[:], sat[:], spt[:])       # s*(1-s)
        Abf = stat.tile([128, 2], f32, tag="Abf", name="Abf")
        nc.vector.tensor_mul(Abf[:], uat[:], spt[:])
        nc.vector.tensor_sub(Abf[:], sat[:], Abf[:])
        Bcol = stat.tile([128, 2], f32, tag="Bcol", name="Bcol")
        nc.vector.tensor_mul(Bcol[:], att[:], spt[:])

        # ---- Phase B ----
        actx.close()

        def xpieces(hw0, width):
            out = []
            pos = hw0
            while pos < hw0 + width:
                for jj, (joff, jsz) in enumerate(CHUNKS):
                    if joff <= pos < joff + jsz:
                        w_ = min(hw0 + width, joff + jsz) - pos
                        out.append((pos - hw0, jj, pos - joff, w_))
                        pos += w_
                        break
                else:
                    raise AssertionError(pos)
            return out

        with tc.tile_pool(name="psB", bufs=2, space="PSUM") as psB:
            for q in range(NQ):
                mode = QMODE[q]
                o = work.tile([128, 2 * CQ], bf16, tag="o", name=f"o{q}",
                              bufs=3)
                sc = work.tile([128, 2 * CQ], bf16, tag="sc",
                               name=f"sc{q}", bufs=2)
                if mode == 'E':
                    pb = psB.tile([128, CQ], f32, tag="psB", name=f"pb{q}")
                    for ss in range(CQ // 512):
                        c0 = q * CQ + ss * 512
                        nc.tensor.matmul(pb[:, ss * 512:(ss + 1) * 512],
                                         on_sb, hrow[0:1, c0:c0 + 512],
                                         start=True, stop=True)
                    for t in range(2):
                        nc.scalar.activation(sc[:, t * CQ:(t + 1) * CQ],
                                             pb[:], AF.Sigmoid,
                                             scale=att[:, t:t + 1])
                else:  # 'P'
                    hbb = work.tile([128, CQ], bf16, tag="hbb",
                                    name=f"hbb{q}", bufs=1)
                    nc.gpsimd.partition_broadcast(
                        hbb[:], hrow[0:1, q * CQ:(q + 1) * CQ])
                    for t in range(2):
                        nc.vector.tensor_scalar(
                            sc[:, t * CQ:(t + 1) * CQ], hbb[:],
                            Bcol[:, t:t + 1], Abf[:, t:t + 1],
                            op0=ALU.mult, op1=ALU.add)
                # wide muls: [128, 2, w] views over chunk pieces
                ov = tview(o[:], CQ)
                sv = tview(sc[:], CQ)
                for (rel, jj, co, w_) in xpieces(q * CQ, CQ):
                    xv = tview(xt[jj][:], CHUNKS[jj][1])
                    nc.vector.tensor_tensor(ov[:, :, rel:rel + w_],
                                            xv[:, :, co:co + w_],
                                            sv[:, :, rel:rel + w_],
                                            op=ALU.mult)
                if q == NQ - 1:
                    for hh in range(2):
                        nc.sync.dma_start(
                            out=outd[:, :, q * CQ + hh * 1024:
                                     q * CQ + (hh + 1) * 1024],
                            in_=ov[:, :, hh * 1024:(hh + 1) * 1024])
                else:
                    nc.sync.dma_start(
                        out=outd[:, :, q * CQ:(q + 1) * CQ], in_=o[:])

    nc.compile()
    return nc


_prog_cache = {}
_TRACE = False      # test harness sets True to collect an NTFF profile
_last_res = None    # BassKernelResults of the most recent run


def kernel(x, dct_w, w1, b1, w2, b2, alpha, lap):
    import ml_dtypes

    x = np.asarray(x, dtype=np.float32)
    dct_w = np.asarray(dct_w, dtype=np.float32)
    w1 = np.asarray(w1, dtype=np.float32)
    b1 = np.asarray(b1, dtype=np.float32)
    w2 = np.asarray(w2, dtype=np.float32)
    b2 = np.asarray(b2, dtype=np.float32)
    alpha = float(np.asarray(alpha))
    lap = np.asarray(lap, dtype=np.float64)

    assert np.allclose(lap[0], lap[2]) and np.allclose(lap[:, 0], lap[:, 2])
    a, b = float(lap[0, 0]), float(lap[0, 1])

    m = dct_w.astype(np.float64).mean(axis=0)           # [C]
    S = np.zeros((H, H), dtype=np.float64)
    for h in range(H):
        S[h, _reflect(h - 1, H)] += 1.0
        S[h, _reflect(h + 1, H)] += 1.0
    from math import comb
    G = (alpha * a) * S
    c24 = 1.0 + alpha * float(lap[1, 1])
    P = c24 * np.eye(H) + 4.0 * G
    Q = (alpha * b) * np.eye(H) + G
    mts = [np.linalg.matrix_power(P, 3 - k) @ np.linalg.matrix_power(Q, k)
           * comb(3, k) for k in range(4)]

    bf16 = ml_dtypes.bfloat16

    # bf16 blob [128, 932]: mv 2 | mt 512 | ones 128 | w1t 32 | w2t 256
    cbh = np.zeros((128, 932), dtype=np.float32)
    cbh[:, 0:2] = np.ascontiguousarray(
        m.astype(np.float32).reshape(2, 128).T)
    for k in range(4):
        cbh[:, 2 + 128 * k:2 + 128 * (k + 1)] = mts[k].T
    cbh[0, 514:642] = 1.0
    cbh[:, 642:674] = w1.T.reshape(2, 128, 16).transpose(1, 0, 2).reshape(
        128, 32)
    cbh[0:16, 674:930] = w2.T
    cbh = cbh.astype(bf16)

    # f32 blob [128, 4]: b2c [128,2] | b1 [16] col2
    cfh = np.zeros((128, 4), dtype=np.float32)
    cfh[:, 0:2] = b2.reshape(2, 128).T
    cfh[0:16, 2] = b1

    key = 0
    if key not in _prog_cache:
        _prog_cache[key] = _build_program(key)
    nc = _prog_cache[key]

    xr = x.reshape(B, 2, 128, HW).transpose(0, 2, 1, 3)
    xr = np.ascontiguousarray(xr).astype(bf16)
    consts = {"cb": cbh, "cf": cfh}
    in_maps = [{"xb": xr[i], **consts} for i in range(N_CORES)]

    from concourse.bass_utils import run_bass_kernel_spmd
    res = run_bass_kernel_spmd(nc, in_maps, list(range(N_CORES)),
                               trace=_TRACE)
    global _last_res
    _last_res = res
    out = np.empty((N_CORES, C, H, W), dtype=np.float32)
    for i in range(N_CORES):
        oi = res.results[i]["out"].astype(np.float32)   # [128, 2, HW]
        out[i] = oi.transpose(1, 0, 2).reshape(C, H, W)
    return out


# revision 13
# speedup vs baseline: 1.4521x; 1.0646x over previous
"""Trainium2 Bass kernel for the HCFDA dense-CNN module (bf16 pipeline, v3).

Math (exact reassociations of the reference):
  1. The 256x256 1x1 DCT conv is only consumed through a channel-mean, so
     temp[b,h,w] = sum_c m[c] * x[b,c,h,w]  with  m = dct_w.mean(axis=0).
  2. The 3 reflect-pad diffusion steps collapse (host-side) into
     T3 = sum_k M_k @ T @ (Sw^T)^k  -> 3 shift-adds + 4 matmuls on device.
  3. SE branch: pooled stats -> two tiny FCs -> sigmoid (branches batched,
     weights in bf16).
  out = x * sigmoid(att[c] * heat[hw]),  heat = sigmoid(T3); exact via ACT
  sigmoid(scale=att) or per-channel Taylor sc ~= A[c] + B[c]*heat (max err
  ~2e-4 at the observed heat range).

Measured op rates (HW bench) that drive the design:
  DVE tensor_tensor bf16: 2x (0.55 ns/col), strided 3D views keep 2x ->
    one op covers both channel halves.  tensor_scalar (AP scalars): 4x
    (0.33 ns/col).  tensor_reduce / accum variants / STT: only 1x.
  ACT: 0.9 ns/col any dtype.  gpsimd partition_broadcast [128,2048]:
    ~3.3 us.  PE matmul: ~0.7 ns/col + ~220 ns.
Hence: pooled max/sum via 2x pairwise fold trees into per-chunk partials
(DVE) + ACT copy-accum for a balanced share of the sums; phase B splits
16 output tiles into 5 exact-q (PE heat-broadcast -> ACT sigmoid -> one
wide DVE mul) + 3 taylor-q (Pool heat-broadcast -> 4x DVE tensor_scalar
-> wide DVE mul), hitting the ~23 us HBM write floor with DVE ~20 us.

DMA layout: x host-interleaved to [128, 2, HW]; 6 chunk loads + 2 packed
const loads issued up-front on the Sync queue (nothing dependent ahead);
Tp scatters + hrow ride the Scalar HWDGE queue; outputs on Sync.

Sharding: pure data parallel, one batch element per NeuronCore (B=8).
"""

import numpy as np
from contextlib import ExitStack

B, C, H, W = 8, 256, 128, 128
HW = H * W           # 16384
N_CORES = 8
H0 = 0.4975          # heat-range center for the Taylor-linear sigmoid

CHUNKS = ((0, 1024), (1024, 3072), (4096, 4096), (8192, 4096),
          (12288, 3584), (15872, 512))

# GEMV psum stages: (hw0, hwlen, psum width)
STAGES = ((0, 8192, 2048), (8192, 4096, 1024), (12288, 4096, 1024))

# sums computed on ACT (copy+accum into `sums` col): (j, t) pairs
SUM_ACT = {(1, 0), (1, 1), (2, 0), (3, 0), (5, 0), (5, 1)}
ACT_SPLIT = 2048     # ACT sum ops chopped to this width (junkA size)

# phase B: 'E' exact (PE pb + ACT sigmoid + DVE mul), 'P' taylor via
# gpsimd partition_broadcast + 4x DVE tensor_scalar + DVE mul
QMODE = ('E', 'E', 'P', 'E', 'P', 'E', 'P', 'E')
CQ = 2048
NQ = HW // CQ


def _reflect(i, n):
    if i < 0:
        return -i
    if i >= n:
        return 2 * (n - 1) - i
    return i


def _build_program(key_unused):
    from concourse import bass, mybir, tile
    from concourse import bacc

    f32 = mybir.dt.float32
    bf16 = mybir.dt.bfloat16
    AF = mybir.ActivationFunctionType
    ALU = mybir.AluOpType
    AX = mybir.AxisListType

    nc = bacc.Bacc("TRN2", target_bir_lowering=False, debug=False,
                   num_devices=N_CORES)

    xb = nc.dram_tensor("xb", [128, 2, HW], bf16, kind="ExternalInput").ap()
    cb = nc.dram_tensor("cb", [128, 932], bf16, kind="ExternalInput").ap()
    cf = nc.dram_tensor("cf", [128, 4], f32, kind="ExternalInput").ap()
    outd = nc.dram_tensor("out", [128, 2, HW], bf16,
                          kind="ExternalOutput").ap()

    def tview(ap_2d, jsz):
        # [128, 2*jsz] chunk tile -> [128, 2, jsz]
        return ap_2d.rearrange("p (t c) -> p t c", t=2)

    with tile.TileContext(nc) as tc, ExitStack() as ctx:
        const = ctx.enter_context(tc.tile_pool(name="const", bufs=1))
        xpool = ctx.enter_context(tc.tile_pool(name="xp", bufs=1))
        work = ctx.enter_context(tc.tile_pool(name="work", bufs=2))
        stat = ctx.enter_context(tc.tile_pool(name="stat", bufs=1))
        actxA = ExitStack()
        psA = [actxA.enter_context(
            tc.tile_pool(name=f"psA{s}", bufs=1, space="PSUM"))
            for s in range(3)]
        actx = ctx.enter_context(ExitStack())

        # ---- loads: first x chunk, consts, rest of x — all on Sync ----
        xt = {}

        def load_chunk(j):
            joff, jsz = CHUNKS[j]
            xt[j] = xpool.tile([128, 2 * jsz], bf16, tag=f"x{j}",
                               name=f"x{j}")
            nc.sync.dma_start(out=xt[j][:], in_=xb[:, :, joff:joff + jsz])

        load_chunk(0)
        cb_sb = const.tile([128, 932], bf16, tag="cb", name="cb")
        nc.sync.dma_start(out=cb_sb[:], in_=cb)
        cf_sb = const.tile([128, 4], f32, tag="cf", name="cf")
        nc.sync.dma_start(out=cf_sb[:], in_=cf)
        for j in range(1, len(CHUNKS)):
            load_chunk(j)

        # const blob views (bf16): mv 2 | mt 4*128 | ones 128 (row0)
        #   | w1t 32 | w2t 256 (rows 0:16)
        m_sb = cb_sb[:, 0:2]
        mt_sb = [cb_sb[:, 2 + 128 * k:2 + 128 * (k + 1)] for k in range(4)]
        on_sb = cb_sb[0:1, 514:642]
        w1_sb = cb_sb[:, 642:674]
        w2_sb = cb_sb[0:16, 674:930]
        # f32 blob: b2c [128,2] | b1 [16,1] col2 | (col3 spare)
        b2c_sb = cf_sb[:, 0:2]
        b1_sb = cf_sb[0:16, 2:3]

        warm = const.tile([1, 2], f32, tag="warm", name="warm")
        nc.scalar.activation(warm[:], cf_sb[0:1, 0:2], AF.Sigmoid)

        sums = stat.tile([128, 2, 8], f32, tag="sums", name="sums")
        junkA = stat.tile([128, 2048], bf16, tag="junkA", name="junkA")
        Tp = [stat.tile([128, W + 2], bf16, tag=f"Tp{i}", name=f"Tp{i}")
              for i in range(4)]
        heat = stat.tile([128, W], bf16, tag="heat", name="heat")
        hrow = stat.tile([1, HW], bf16, tag="hrow", name="hrow")
        stg = [stat.tile([128, STAGES[s][2]], bf16, tag=f"stg{s}",
                         name=f"stg{s}") for s in range(3)]
        pss = [psA[s].tile([128, STAGES[s][2]], f32, tag=f"ps{s}",
                           name=f"ps{s}") for s in range(3)]

        def stage_of(hw0):
            for s, (s0, slen, sw) in enumerate(STAGES):
                if s0 <= hw0 < s0 + slen:
                    return s, s0, sw
            raise AssertionError(hw0)

        def emit_gemv(j):
            joff, jsz = CHUNKS[j]
            for G0 in range(joff, joff + jsz, 512):
                s, s0, sw = stage_of(G0)
                gl = (G0 - s0) // 512
                ncol = sw // 512
                k, g = gl // ncol, gl % ncol
                dst = pss[s][32 * k:32 * k + 1, 512 * g:512 * g + 512]
                co = G0 - joff
                for t in range(2):
                    nc.tensor.matmul(
                        dst, m_sb[:, t:t + 1],
                        xt[j][:, t * jsz + co:t * jsz + co + 512],
                        start=(t == 0), stop=(t == 1),
                        tile_position=(0, 32 * k))
            for s, (s0, slen, sw) in enumerate(STAGES):
                if s0 + slen == joff + jsz:
                    nc.scalar.activation(stg[s][:], pss[s][:], AF.Copy)
                    r0 = s0 // 128
                    nr = slen // 128
                    nc.scalar.dma_start(
                        out=Tp[0][r0:r0 + nr, 1:W + 1],
                        in_=stg[s][0:128:32, :])

        # running stat accumulators [128, 2, 1024] bf16; per-chunk
        # folds merge in immediately (no serial merge tail).
        nmx = stat.tile([128, 2, 1024], bf16, tag="nmx", name="nmx")
        qs = stat.tile([128, 2, 1024], bf16, tag="qs", name="qs")
        scol = {0: 0, 1: 0}     # next free `sums` column per t

        def act_sum(j, t, jsz):
            xs = xt[j][:, t * jsz:(t + 1) * jsz]
            for o0 in range(0, jsz, ACT_SPLIT):
                w_ = min(ACT_SPLIT, jsz - o0)
                c = scol[t]
                scol[t] += 1
                nc.scalar.activation(
                    junkA[:, 0:w_], xs[:, o0:o0 + w_], AF.Copy,
                    accum_out=sums[:, t:t + 1, c:c + 1])

        def fold_tree(x3, jsz, op, tag):
            """fold [128,2,jsz] -> returns [128,2,1024] AP (2x DVE)."""
            f = work.tile([128, 2, 1024], bf16, tag=tag, name=f"{tag}_x",
                          bufs=2)
            if jsz == 3072:
                nc.vector.tensor_tensor(f[:], x3[:, :, 0:1024],
                                        x3[:, :, 1024:2048], op=op)
                nc.vector.tensor_tensor(f[:], f[:], x3[:, :, 2048:3072],
                                        op=op)
            elif jsz == 4096:
                f2 = work.tile([128, 2, 2048], bf16, tag=tag + "w",
                               name=f"{tag}w_x", bufs=1)
                nc.vector.tensor_tensor(f2[:], x3[:, :, 0:2048],
                                        x3[:, :, 2048:4096], op=op)
                nc.vector.tensor_tensor(f[:], f2[:, :, 0:1024],
                                        f2[:, :, 1024:2048], op=op)
            elif jsz == 3584:
                nc.vector.tensor_tensor(f[:], x3[:, :, 0:1024],
                                        x3[:, :, 1024:2048], op=op)
                nc.vector.tensor_tensor(f[:], f[:], x3[:, :, 2048:3072],
                                        op=op)
                nc.vector.tensor_tensor(f[:, :, 0:512], f[:, :, 0:512],
                                        x3[:, :, 3072:3584], op=op)
            else:
                raise AssertionError(jsz)
            return f

        def emit_stats(j):
            joff, jsz = CHUNKS[j]
            x3 = tview(xt[j][:], jsz)
            for t in range(2):
                if (j, t) in SUM_ACT:
                    act_sum(j, t, jsz)
            if j == 0:
                return          # x0 itself seeds nmx/qs at j==1
            if j == 5:
                return          # handled in finalize
            f = fold_tree(x3, jsz, ALU.max, "mf")
            if j == 1:
                nc.vector.tensor_tensor(nmx[:], tview(xt[0][:], 1024),
                                        f[:], op=ALU.max)
            else:
                nc.vector.tensor_tensor(nmx[:], nmx[:], f[:], op=ALU.max)
            # DVE sum partials: (2,t1), (3,t1), (4,both)
            if j in (2, 3):
                x1 = x3[:, 1:2, :]
                sp = work.tile([128, 1, 1024], bf16, tag="sf1",
                               name=f"sf1_{j}", bufs=2)
                f2 = work.tile([128, 1, 2048], bf16, tag="sf1w",
                               name=f"sf1w_{j}", bufs=1)
                nc.vector.tensor_tensor(f2[:], x1[:, :, 0:2048],
                                        x1[:, :, 2048:4096], op=ALU.add)
                nc.vector.tensor_tensor(sp[:], f2[:, :, 0:1024],
                                        f2[:, :, 1024:2048], op=ALU.add)
                if j == 2:
                    nc.vector.tensor_tensor(qs[:, 1:2, :],
                                            tview(xt[0][:], 1024)[:, 1:2, :],
                                            sp[:], op=ALU.add)
                else:
                    nc.vector.tensor_tensor(qs[:, 1:2, :], qs[:, 1:2, :],
                                            sp[:], op=ALU.add)
            elif j == 4:
                sp = fold_tree(x3, jsz, ALU.add, "sf")
                nc.vector.tensor_tensor(qs[:, 1:2, :], qs[:, 1:2, :],
                                        sp[:, 1:2, :], op=ALU.add)
                nc.vector.tensor_tensor(qs[:, 0:1, :],
                                        tview(xt[0][:], 1024)[:, 0:1, :],
                                        sp[:, 0:1, :], op=ALU.add)

        for j in range(len(CHUNKS)):
            with tc.high_priority():
                emit_gemv(j)
            emit_stats(j)

        # ---- finalize: big reduces overlap the j5 transfer ----
        ymax = stat.tile([128, 2], f32, tag="ymax", name="ymax")
        ysum = stat.tile([128, 2], f32, tag="ysum", name="ysum")
        m1 = stat.tile([128, 2], f32, tag="m1", name="m1")
        nc.vector.tensor_reduce(m1[:], nmx[:], axis=AX.X, op=ALU.max)
        for t in range(2):
            c = scol[t]
            scol[t] += 1
            nc.scalar.activation(junkA[:, 0:1024], qs[:, t, :], AF.Copy,
                                 accum_out=sums[:, t:t + 1, c:c + 1])
        x5 = tview(xt[5][:], 512)
        for t in range(2):
            j5m = stat.tile([128, 1], f32, tag=f"j5m{t}", name=f"j5m{t}")
            nc.vector.tensor_reduce(j5m[:], x5[:, t:t + 1, :], axis=AX.XY,
                                    op=ALU.max)
            nc.vector.tensor_tensor(ymax[:, t:t + 1], m1[:, t:t + 1],
                                    j5m[:], op=ALU.max)
            nc.vector.tensor_reduce(ysum[:, t:t + 1],
                                    sums[:, t:t + 1, 0:scol[t]],
                                    axis=AX.XY, op=ALU.add)

        # ---- diffusion on Tp + heat (scalar-queue DMAs) ----
        actxA.close()
        psD = actx.enter_context(tc.tile_pool(name="psD", bufs=1,
                                              space="PSUM"))
        psF = actx.enter_context(tc.tile_pool(name="psF", bufs=1,
                                              space="PSUM"))
        with tc.high_priority():
            nc.vector.tensor_copy(Tp[0][:, 0:1], Tp[0][:, 2:3])
            nc.vector.tensor_copy(Tp[0][:, W + 1:W + 2], Tp[0][:, W - 1:W])
            pd3 = psD.tile([128, W], f32, tag="psD", name="psD")
            for k in range(4):
                nc.tensor.matmul(pd3[:], mt_sb[k], Tp[k][:, 1:W + 1],
                                 start=(k == 0), stop=(k == 3))
                if k < 3:
                    nxt = Tp[k + 1]
                    nc.vector.tensor_add(nxt[:, 1:W + 1], Tp[k][:, 0:W],
                                         Tp[k][:, 2:W + 2])
                    nc.vector.tensor_copy(nxt[:, 0:1], nxt[:, 2:3])
                    nc.vector.tensor_copy(nxt[:, W + 1:W + 2],
                                          nxt[:, W - 1:W])
            nc.scalar.activation(heat[:], pd3[:], AF.Sigmoid)
            nc.scalar.dma_start(out=hrow[0:1, 0:HW // 2], in_=heat[0:64, :])
            nc.scalar.dma_start(out=hrow[0:1, HW // 2:HW],
                                in_=heat[64:128, :])

        # ---- batched SE FC chain (bf16 weights) ----
        att = stat.tile([128, 2], f32, tag="att", name="att")
        Yb = stat.tile([128, 4], bf16, tag="Yb", name="Yb")
        for t in range(2):
            nc.vector.tensor_scalar_mul(Yb[:, 2 * t:2 * t + 1],
                                        ysum[:, t:t + 1], 1.0 / HW)
            nc.vector.tensor_copy(Yb[:, 2 * t + 1:2 * t + 2],
                                  ymax[:, t:t + 1])
        ph = psF.tile([16, 2], f32, tag="psF", name="ph")
        nc.tensor.matmul(ph[:], w1_sb[:, 0:16], Yb[:, 0:2],
                         start=True, stop=False)
        nc.tensor.matmul(ph[:], w1_sb[:, 16:32], Yb[:, 2:4],
                         start=False, stop=True)
        hb = stat.tile([16, 2], bf16, tag="hb", name="hb")
        nc.scalar.activation(hb[:], ph[:], AF.Relu, bias=b1_sb)
        for t in range(2):
            pa = psF.tile([128, 2], f32, tag="psFa", name=f"pa{t}")
            nc.tensor.matmul(pa[:], w2_sb[:, 128 * t:128 * (t + 1)], hb[:],
                             start=True, stop=True)
            sg = stat.tile([128, 2], f32, tag=f"sg{t}", name=f"sg{t}")
            nc.scalar.activation(sg[:], pa[:], AF.Sigmoid,
                                 bias=b2c_sb[:, t:t + 1])
            nc.vector.tensor_add(att[:, t:t + 1], sg[:, 0:1], sg[:, 1:2])

        # Taylor coeffs (column form only): sc ~= A + B*heat
        uat = stat.tile([128, 2], f32, tag="uat", name="uat")
        nc.vector.tensor_scalar_mul(uat[:], att[:], H0)
        sat = stat.tile([128, 2], f32, tag="sat", name="sat")
        nc.scalar.activation(sat[:], uat[:], AF.Sigmoid)
        spt = stat.tile([128, 2], f32, tag="spt", name="spt")
        nc.vector.tensor_mul(spt[:], sat[:], sat[:])
        nc.vector.tensor_sub(spt[:], sat[:], spt[:])       # s*(1-s)
        Abf = stat.tile([128, 2], f32, tag="Abf", name="Abf")
        nc.vector.tensor_mul(Abf[:], uat[:], spt[:])
        nc.vector.tensor_sub(Abf[:], sat[:], Abf[:])
        Bcol = stat.tile([128, 2], f32, tag="Bcol", name="Bcol")
        nc.vector.tensor_mul(Bcol[:], att[:], spt[:])

        # ---- Phase B ----
        actx.close()

        def xpieces(hw0, width):
            out = []
            pos = hw0
            while pos < hw0 + width:
                for jj, (joff, jsz) in enumerate(CHUNKS):
                    if joff <= pos < joff + jsz:
                        w_ = min(hw0 + width, joff + jsz) - pos
                        out.append((pos - hw0, jj, pos - joff, w_))
                        pos += w_
                        break
                else:
                    raise AssertionError(pos)
            return out

        with tc.tile_pool(name="psB", bufs=2, space="PSUM") as psB:
            # pre-issue the Pool heat-broadcasts for the 'P' qs so they
            # pipeline under the early 'E' work (hbb slots recycle 2-deep)
            hbbs = {}
            for q in range(NQ):
                if QMODE[q] == 'P':
                    hbbs[q] = work.tile([128, CQ], bf16, tag="hbb",
                                        name=f"hbb{q}", bufs=2)
                    nc.gpsimd.partition_broadcast(
                        hbbs[q][:], hrow[0:1, q * CQ:(q + 1) * CQ])
            for q in range(NQ):
                mode = QMODE[q]
                o = work.tile([128, 2 * CQ], bf16, tag="o", name=f"o{q}",
                              bufs=3)
                sc = work.tile([128, 2 * CQ], bf16, tag="sc",
                               name=f"sc{q}", bufs=2)
                if mode == 'E':
                    pb = psB.tile([128, CQ], f32, tag="psB", name=f"pb{q}")
                    for ss in range(CQ // 512):
                        c0 = q * CQ + ss * 512
                        nc.tensor.matmul(pb[:, ss * 512:(ss + 1) * 512],
                                         on_sb, hrow[0:1, c0:c0 + 512],
                                         start=True, stop=True)
                    for t in range(2):
                        nc.scalar.activation(sc[:, t * CQ:(t + 1) * CQ],
                                             pb[:], AF.Sigmoid,
                                             scale=att[:, t:t + 1])
                else:  # 'P'
                    hbb = hbbs[q]
                    for t in range(2):
                        nc.vector.tensor_scalar(
                            sc[:, t * CQ:(t + 1) * CQ], hbb[:],
                            Bcol[:, t:t + 1], Abf[:, t:t + 1],
                            op0=ALU.mult, op1=ALU.add)
                # wide muls: [128, 2, w] views over chunk pieces
                ov = tview(o[:], CQ)
                sv = tview(sc[:], CQ)
                for (rel, jj, co, w_) in xpieces(q * CQ, CQ):
                    xv = tview(xt[jj][:], CHUNKS[jj][1])
                    nc.vector.tensor_tensor(ov[:, :, rel:rel + w_],
                                            xv[:, :, co:co + w_],
                                            sv[:, :, rel:rel + w_],
                                            op=ALU.mult)
                if q == NQ - 1:
                    for hh in range(2):
                        nc.sync.dma_start(
                            out=outd[:, :, q * CQ + hh * 1024:
                                     q * CQ + (hh + 1) * 1024],
                            in_=ov[:, :, hh * 1024:(hh + 1) * 1024])
                else:
                    nc.sync.dma_start(
                        out=outd[:, :, q * CQ:(q + 1) * CQ], in_=o[:])

    nc.compile()
    return nc


_prog_cache = {}
_TRACE = False      # test harness sets True to collect an NTFF profile
_last_res = None    # BassKernelResults of the most recent run


def kernel(x, dct_w, w1, b1, w2, b2, alpha, lap):
    import ml_dtypes

    x = np.asarray(x, dtype=np.float32)
    dct_w = np.asarray(dct_w, dtype=np.float32)
    w1 = np.asarray(w1, dtype=np.float32)
    b1 = np.asarray(b1, dtype=np.float32)
    w2 = np.asarray(w2, dtype=np.float32)
    b2 = np.asarray(b2, dtype=np.float32)
    alpha = float(np.asarray(alpha))
    lap = np.asarray(lap, dtype=np.float64)

    assert np.allclose(lap[0], lap[2]) and np.allclose(lap[:, 0], lap[:, 2])
    a, b = float(lap[0, 0]), float(lap[0, 1])

    m = dct_w.astype(np.float64).mean(axis=0)           # [C]
    S = np.zeros((H, H), dtype=np.float64)
    for h in range(H):
        S[h, _reflect(h - 1, H)] += 1.0
        S[h, _reflect(h + 1, H)] += 1.0
    from math import comb
    G = (alpha * a) * S
    c24 = 1.0 + alpha * float(lap[1, 1])
    P = c24 * np.eye(H) + 4.0 * G
    Q = (alpha * b) * np.eye(H) + G
    mts = [np.linalg.matrix_power(P, 3 - k) @ np.linalg.matrix_power(Q, k)
           * comb(3, k) for k in range(4)]

    bf16 = ml_dtypes.bfloat16

    # bf16 blob [128, 932]: mv 2 | mt 512 | ones 128 | w1t 32 | w2t 256
    cbh = np.zeros((128, 932), dtype=np.float32)
    cbh[:, 0:2] = np.ascontiguousarray(
        m.astype(np.float32).reshape(2, 128).T)
    for k in range(4):
        cbh[:, 2 + 128 * k:2 + 128 * (k + 1)] = mts[k].T
    cbh[0, 514:642] = 1.0
    cbh[:, 642:674] = w1.T.reshape(2, 128, 16).transpose(1, 0, 2).reshape(
        128, 32)
    cbh[0:16, 674:930] = w2.T
    cbh = cbh.astype(bf16)

    # f32 blob [128, 4]: b2c [128,2] | b1 [16] col2
    cfh = np.zeros((128, 4), dtype=np.float32)
    cfh[:, 0:2] = b2.reshape(2, 128).T
    cfh[0:16, 2] = b1

    key = 0
    if key not in _prog_cache:
        _prog_cache[key] = _build_program(key)
    nc = _prog_cache[key]

    xr = x.reshape(B, 2, 128, HW).transpose(0, 2, 1, 3)
    xr = np.ascontiguousarray(xr).astype(bf16)
    consts = {"cb": cbh, "cf": cfh}
    in_maps = [{"xb": xr[i], **consts} for i in range(N_CORES)]

    from concourse.bass_utils import run_bass_kernel_spmd
    res = run_bass_kernel_spmd(nc, in_maps, list(range(N_CORES)),
                               trace=_TRACE)
    global _last_res
    _last_res = res
    out = np.empty((N_CORES, C, H, W), dtype=np.float32)
    for i in range(N_CORES):
        oi = res.results[i]["out"].astype(np.float32)   # [128, 2, HW]
        out[i] = oi.transpose(1, 0, 2).reshape(C, H, W)
    return out


# revision 14
# speedup vs baseline: 1.5218x; 1.0480x over previous
"""Trainium2 Bass kernel for the HCFDA dense-CNN module (bf16 pipeline, v3).

Math (exact reassociations of the reference):
  1. The 256x256 1x1 DCT conv is only consumed through a channel-mean, so
     temp[b,h,w] = sum_c m[c] * x[b,c,h,w]  with  m = dct_w.mean(axis=0).
  2. The 3 reflect-pad diffusion steps collapse (host-side) into
     T3 = sum_k M_k @ T @ (Sw^T)^k  -> 3 shift-adds + 4 matmuls on device.
  3. SE branch: pooled stats -> two tiny FCs -> sigmoid (branches batched,
     weights in bf16).
  out = x * sigmoid(att[c] * heat[hw]),  heat = sigmoid(T3); exact via ACT
  sigmoid(scale=att) or per-channel Taylor sc ~= A[c] + B[c]*heat (max err
  ~2e-4 at the observed heat range).

Measured op rates (HW bench) that drive the design:
  DVE tensor_tensor bf16: 2x (0.55 ns/col), strided 3D views keep 2x ->
    one op covers both channel halves.  tensor_scalar (AP scalars): 4x
    (0.33 ns/col).  tensor_reduce / accum variants / STT: only 1x.
  ACT: 0.9 ns/col any dtype.  gpsimd partition_broadcast [128,2048]:
    ~3.3 us.  PE matmul: ~0.7 ns/col + ~220 ns.
Hence: pooled max/sum via 2x pairwise fold trees into per-chunk partials
(DVE) + ACT copy-accum for a balanced share of the sums; phase B splits
16 output tiles into 5 exact-q (PE heat-broadcast -> ACT sigmoid -> one
wide DVE mul) + 3 taylor-q (Pool heat-broadcast -> 4x DVE tensor_scalar
-> wide DVE mul), hitting the ~23 us HBM write floor with DVE ~20 us.

DMA layout: x host-interleaved to [128, 2, HW]; 6 chunk loads + 2 packed
const loads issued up-front on the Sync queue (nothing dependent ahead);
Tp scatters + hrow ride the Scalar HWDGE queue; outputs on Sync.

Sharding: pure data parallel, one batch element per NeuronCore (B=8).
"""

import numpy as np
from contextlib import ExitStack

B, C, H, W = 8, 256, 128, 128
HW = H * W           # 16384
N_CORES = 8
H0 = 0.4975          # heat-range center for the Taylor-linear sigmoid

CHUNKS = ((0, 1024), (1024, 3072), (4096, 4096), (8192, 4096),
          (12288, 3584), (15872, 512))

# GEMV psum stages: (hw0, hwlen, psum width)
STAGES = ((0, 8192, 2048), (8192, 4096, 1024), (12288, 4096, 1024))

# sums computed on ACT (copy+accum into `sums` col): (j, t) pairs
SUM_ACT = {(1, 0), (1, 1), (2, 0), (3, 0), (5, 0), (5, 1)}
ACT_SPLIT = 2048     # ACT sum ops chopped to this width (junkA size)

# phase B: 'E' exact (PE pb + ACT sigmoid + DVE mul), 'P' taylor via
# gpsimd partition_broadcast + 4x DVE tensor_scalar + DVE mul
QMODE = ('E', 'C', 'E', 'C', 'E', 'C', 'E', 'C')
CQ = 2048
NQ = HW // CQ


def _reflect(i, n):
    if i < 0:
        return -i
    if i >= n:
        return 2 * (n - 1) - i
    return i


def _build_program(key_unused):
    from concourse import bass, mybir, tile
    from concourse import bacc

    f32 = mybir.dt.float32
    bf16 = mybir.dt.bfloat16
    AF = mybir.ActivationFunctionType
    ALU = mybir.AluOpType
    AX = mybir.AxisListType

    nc = bacc.Bacc("TRN2", target_bir_lowering=False, debug=False,
                   num_devices=N_CORES)

    xb = nc.dram_tensor("xb", [128, 2, HW], bf16, kind="ExternalInput").ap()
    cb = nc.dram_tensor("cb", [128, 932], bf16, kind="ExternalInput").ap()
    cf = nc.dram_tensor("cf", [128, 4], f32, kind="ExternalInput").ap()
    outd = nc.dram_tensor("out", [128, 2, HW], bf16,
                          kind="ExternalOutput").ap()

    def tview(ap_2d, jsz):
        # [128, 2*jsz] chunk tile -> [128, 2, jsz]
        return ap_2d.rearrange("p (t c) -> p t c", t=2)

    with tile.TileContext(nc) as tc, ExitStack() as ctx:
        const = ctx.enter_context(tc.tile_pool(name="const", bufs=1))
        xpool = ctx.enter_context(tc.tile_pool(name="xp", bufs=1))
        work = ctx.enter_context(tc.tile_pool(name="work", bufs=2))
        stat = ctx.enter_context(tc.tile_pool(name="stat", bufs=1))
        actxA = ExitStack()
        psA = [actxA.enter_context(
            tc.tile_pool(name=f"psA{s}", bufs=1, space="PSUM"))
            for s in range(3)]
        actx = ctx.enter_context(ExitStack())

        # ---- loads: first x chunk, consts, rest of x — all on Sync ----
        xt = {}

        def load_chunk(j):
            joff, jsz = CHUNKS[j]
            xt[j] = xpool.tile([128, 2 * jsz], bf16, tag=f"x{j}",
                               name=f"x{j}")
            nc.sync.dma_start(out=xt[j][:], in_=xb[:, :, joff:joff + jsz])

        load_chunk(0)
        cb_sb = const.tile([128, 932], bf16, tag="cb", name="cb")
        nc.sync.dma_start(out=cb_sb[:], in_=cb)
        cf_sb = const.tile([128, 4], f32, tag="cf", name="cf")
        nc.sync.dma_start(out=cf_sb[:], in_=cf)
        for j in range(1, len(CHUNKS)):
            load_chunk(j)

        # const blob views (bf16): mv 2 | mt 4*128 | ones 128 (row0)
        #   | w1t 32 | w2t 256 (rows 0:16)
        m_sb = cb_sb[:, 0:2]
        mt_sb = [cb_sb[:, 2 + 128 * k:2 + 128 * (k + 1)] for k in range(4)]
        on_sb = cb_sb[0:1, 514:642]
        w1_sb = cb_sb[:, 642:674]
        w2_sb = cb_sb[0:16, 674:930]
        # f32 blob: b2c [128,2] | b1 [16,1] col2 | (col3 spare)
        b2c_sb = cf_sb[:, 0:2]
        b1_sb = cf_sb[0:16, 2:3]

        warm = const.tile([1, 2], f32, tag="warm", name="warm")
        nc.scalar.activation(warm[:], cf_sb[0:1, 0:2], AF.Sigmoid)

        sums = stat.tile([128, 2, 8], f32, tag="sums", name="sums")
        junkA = stat.tile([128, 2048], bf16, tag="junkA", name="junkA")
        Tp = [stat.tile([128, W + 2], bf16, tag=f"Tp{i}", name=f"Tp{i}")
              for i in range(4)]
        heat = stat.tile([128, W], bf16, tag="heat", name="heat")
        hrow = stat.tile([1, HW], bf16, tag="hrow", name="hrow")
        stg = [stat.tile([128, STAGES[s][2]], bf16, tag=f"stg{s}",
                         name=f"stg{s}") for s in range(3)]
        pss = [psA[s].tile([128, STAGES[s][2]], f32, tag=f"ps{s}",
                           name=f"ps{s}") for s in range(3)]

        def stage_of(hw0):
            for s, (s0, slen, sw) in enumerate(STAGES):
                if s0 <= hw0 < s0 + slen:
                    return s, s0, sw
            raise AssertionError(hw0)

        def emit_gemv(j):
            joff, jsz = CHUNKS[j]
            for G0 in range(joff, joff + jsz, 512):
                s, s0, sw = stage_of(G0)
                gl = (G0 - s0) // 512
                ncol = sw // 512
                k, g = gl // ncol, gl % ncol
                dst = pss[s][32 * k:32 * k + 1, 512 * g:512 * g + 512]
                co = G0 - joff
                for t in range(2):
                    nc.tensor.matmul(
                        dst, m_sb[:, t:t + 1],
                        xt[j][:, t * jsz + co:t * jsz + co + 512],
                        start=(t == 0), stop=(t == 1),
                        tile_position=(0, 32 * k))
            for s, (s0, slen, sw) in enumerate(STAGES):
                if s0 + slen == joff + jsz:
                    nc.scalar.activation(stg[s][:], pss[s][:], AF.Copy)
                    r0 = s0 // 128
                    nr = slen // 128
                    nc.scalar.dma_start(
                        out=Tp[0][r0:r0 + nr, 1:W + 1],
                        in_=stg[s][0:128:32, :])

        # running stat accumulators [128, 2, 1024] bf16; per-chunk
        # folds merge in immediately (no serial merge tail).
        nmx = stat.tile([128, 2, 1024], bf16, tag="nmx", name="nmx")
        qs = stat.tile([128, 2, 1024], bf16, tag="qs", name="qs")
        scol = {0: 0, 1: 0}     # next free `sums` column per t

        def act_sum(j, t, jsz):
            xs = xt[j][:, t * jsz:(t + 1) * jsz]
            for o0 in range(0, jsz, ACT_SPLIT):
                w_ = min(ACT_SPLIT, jsz - o0)
                c = scol[t]
                scol[t] += 1
                nc.scalar.activation(
                    junkA[:, 0:w_], xs[:, o0:o0 + w_], AF.Copy,
                    accum_out=sums[:, t:t + 1, c:c + 1])

        def fold_tree(x3, jsz, op, tag):
            """fold [128,2,jsz] -> returns [128,2,1024] AP (2x DVE)."""
            f = work.tile([128, 2, 1024], bf16, tag=tag, name=f"{tag}_x",
                          bufs=2)
            if jsz == 3072:
                nc.vector.tensor_tensor(f[:], x3[:, :, 0:1024],
                                        x3[:, :, 1024:2048], op=op)
                nc.vector.tensor_tensor(f[:], f[:], x3[:, :, 2048:3072],
                                        op=op)
            elif jsz == 4096:
                f2 = work.tile([128, 2, 2048], bf16, tag=tag + "w",
                               name=f"{tag}w_x", bufs=1)
                nc.vector.tensor_tensor(f2[:], x3[:, :, 0:2048],
                                        x3[:, :, 2048:4096], op=op)
                nc.vector.tensor_tensor(f[:], f2[:, :, 0:1024],
                                        f2[:, :, 1024:2048], op=op)
            elif jsz == 3584:
                nc.vector.tensor_tensor(f[:], x3[:, :, 0:1024],
                                        x3[:, :, 1024:2048], op=op)
                nc.vector.tensor_tensor(f[:], f[:], x3[:, :, 2048:3072],
                                        op=op)
                nc.vector.tensor_tensor(f[:, :, 0:512], f[:, :, 0:512],
                                        x3[:, :, 3072:3584], op=op)
            else:
                raise AssertionError(jsz)
            return f

        def emit_stats(j):
            joff, jsz = CHUNKS[j]
            x3 = tview(xt[j][:], jsz)
            for t in range(2):
                if (j, t) in SUM_ACT:
                    act_sum(j, t, jsz)
            if j == 0:
                return          # x0 itself seeds nmx/qs at j==1
            if j == 5:
                return          # handled in finalize
            f = fold_tree(x3, jsz, ALU.max, "mf")
            if j == 1:
                nc.vector.tensor_tensor(nmx[:], tview(xt[0][:], 1024),
                                        f[:], op=ALU.max)
            else:
                nc.vector.tensor_tensor(nmx[:], nmx[:], f[:], op=ALU.max)
            # DVE sum partials: (2,t1), (3,t1), (4,both)
            if j in (2, 3):
                x1 = x3[:, 1:2, :]
                sp = work.tile([128, 1, 1024], bf16, tag="sf1",
                               name=f"sf1_{j}", bufs=2)
                f2 = work.tile([128, 1, 2048], bf16, tag="sf1w",
                               name=f"sf1w_{j}", bufs=1)
                nc.vector.tensor_tensor(f2[:], x1[:, :, 0:2048],
                                        x1[:, :, 2048:4096], op=ALU.add)
                nc.vector.tensor_tensor(sp[:], f2[:, :, 0:1024],
                                        f2[:, :, 1024:2048], op=ALU.add)
                if j == 2:
                    nc.vector.tensor_tensor(qs[:, 1:2, :],
                                            tview(xt[0][:], 1024)[:, 1:2, :],
                                            sp[:], op=ALU.add)
                else:
                    nc.vector.tensor_tensor(qs[:, 1:2, :], qs[:, 1:2, :],
                                            sp[:], op=ALU.add)
            elif j == 4:
                sp = fold_tree(x3, jsz, ALU.add, "sf")
                nc.vector.tensor_tensor(qs[:, 1:2, :], qs[:, 1:2, :],
                                        sp[:, 1:2, :], op=ALU.add)
                nc.vector.tensor_tensor(qs[:, 0:1, :],
                                        tview(xt[0][:], 1024)[:, 0:1, :],
                                        sp[:, 0:1, :], op=ALU.add)

        for j in range(len(CHUNKS)):
            with tc.high_priority():
                emit_gemv(j)
            emit_stats(j)

        # ---- finalize: big reduces overlap the j5 transfer ----
        ymax = stat.tile([128, 2], f32, tag="ymax", name="ymax")
        ysum = stat.tile([128, 2], f32, tag="ysum", name="ysum")
        m1 = stat.tile([128, 2], f32, tag="m1", name="m1")
        nc.vector.tensor_reduce(m1[:], nmx[:], axis=AX.X, op=ALU.max)
        for t in range(2):
            c = scol[t]
            scol[t] += 1
            nc.scalar.activation(junkA[:, 0:1024], qs[:, t, :], AF.Copy,
                                 accum_out=sums[:, t:t + 1, c:c + 1])
        x5 = tview(xt[5][:], 512)
        for t in range(2):
            j5m = stat.tile([128, 1], f32, tag=f"j5m{t}", name=f"j5m{t}")
            nc.vector.tensor_reduce(j5m[:], x5[:, t:t + 1, :], axis=AX.XY,
                                    op=ALU.max)
            nc.vector.tensor_tensor(ymax[:, t:t + 1], m1[:, t:t + 1],
                                    j5m[:], op=ALU.max)
            nc.vector.tensor_reduce(ysum[:, t:t + 1],
                                    sums[:, t:t + 1, 0:scol[t]],
                                    axis=AX.XY, op=ALU.add)

        # ---- diffusion on Tp + heat (scalar-queue DMAs) ----
        actxA.close()
        psD = actx.enter_context(tc.tile_pool(name="psD", bufs=1,
                                              space="PSUM"))
        psF = actx.enter_context(tc.tile_pool(name="psF", bufs=1,
                                              space="PSUM"))
        with tc.high_priority():
            nc.vector.tensor_copy(Tp[0][:, 0:1], Tp[0][:, 2:3])
            nc.vector.tensor_copy(Tp[0][:, W + 1:W + 2], Tp[0][:, W - 1:W])
            pd3 = psD.tile([128, W], f32, tag="psD", name="psD")
            for k in range(4):
                nc.tensor.matmul(pd3[:], mt_sb[k], Tp[k][:, 1:W + 1],
                                 start=(k == 0), stop=(k == 3))
                if k < 3:
                    nxt = Tp[k + 1]
                    nc.vector.tensor_add(nxt[:, 1:W + 1], Tp[k][:, 0:W],
                                         Tp[k][:, 2:W + 2])
                    nc.vector.tensor_copy(nxt[:, 0:1], nxt[:, 2:3])
                    nc.vector.tensor_copy(nxt[:, W + 1:W + 2],
                                          nxt[:, W - 1:W])
            nc.scalar.activation(heat[:], pd3[:], AF.Sigmoid)
            nc.scalar.dma_start(out=hrow[0:1, 0:HW // 2], in_=heat[0:64, :])
            nc.scalar.dma_start(out=hrow[0:1, HW // 2:HW],
                                in_=heat[64:128, :])

        # ---- batched SE FC chain (bf16 weights) ----
        att = stat.tile([128, 2], f32, tag="att", name="att")
        Yb = stat.tile([128, 4], bf16, tag="Yb", name="Yb")
        for t in range(2):
            nc.vector.tensor_scalar_mul(Yb[:, 2 * t:2 * t + 1],
                                        ysum[:, t:t + 1], 1.0 / HW)
            nc.vector.tensor_copy(Yb[:, 2 * t + 1:2 * t + 2],
                                  ymax[:, t:t + 1])
        ph = psF.tile([16, 2], f32, tag="psF", name="ph")
        nc.tensor.matmul(ph[:], w1_sb[:, 0:16], Yb[:, 0:2],
                         start=True, stop=False)
        nc.tensor.matmul(ph[:], w1_sb[:, 16:32], Yb[:, 2:4],
                         start=False, stop=True)
        hb = stat.tile([16, 2], bf16, tag="hb", name="hb")
        nc.scalar.activation(hb[:], ph[:], AF.Relu, bias=b1_sb)
        for t in range(2):
            pa = psF.tile([128, 2], f32, tag="psFa", name=f"pa{t}")
            nc.tensor.matmul(pa[:], w2_sb[:, 128 * t:128 * (t + 1)], hb[:],
                             start=True, stop=True)
            sg = stat.tile([128, 2], f32, tag=f"sg{t}", name=f"sg{t}")
            nc.scalar.activation(sg[:], pa[:], AF.Sigmoid,
                                 bias=b2c_sb[:, t:t + 1])
            nc.vector.tensor_add(att[:, t:t + 1], sg[:, 0:1], sg[:, 1:2])

        # Taylor coeffs (column form only): sc ~= A + B*heat
        uat = stat.tile([128, 2], f32, tag="uat", name="uat")
        nc.vector.tensor_scalar_mul(uat[:], att[:], H0)
        sat = stat.tile([128, 2], f32, tag="sat", name="sat")
        nc.scalar.activation(sat[:], uat[:], AF.Sigmoid)
        spt = stat.tile([128, 2], f32, tag="spt", name="spt")
        nc.vector.tensor_mul(spt[:], sat[:], sat[:])
        nc.vector.tensor_sub(spt[:], sat[:], spt[:])       # s*(1-s)
        Abf = stat.tile([128, 2], f32, tag="Abf", name="Abf")
        nc.vector.tensor_mul(Abf[:], uat[:], spt[:])
        nc.vector.tensor_sub(Abf[:], sat[:], Abf[:])
        Bcol = stat.tile([128, 2], f32, tag="Bcol", name="Bcol")
        nc.vector.tensor_mul(Bcol[:], att[:], spt[:])

        # ---- Phase B ----
        actx.close()

        def xpieces(hw0, width):
            out = []
            pos = hw0
            while pos < hw0 + width:
                for jj, (joff, jsz) in enumerate(CHUNKS):
                    if joff <= pos < joff + jsz:
                        w_ = min(hw0 + width, joff + jsz) - pos
                        out.append((pos - hw0, jj, pos - joff, w_))
                        pos += w_
                        break
                else:
                    raise AssertionError(pos)
            return out

        with tc.tile_pool(name="psB", bufs=2, space="PSUM") as psB:
            for q in range(NQ):
                mode = QMODE[q]
                o = work.tile([128, 2 * CQ], bf16, tag="o", name=f"o{q}",
                              bufs=3)
                sc = work.tile([128, 2 * CQ], bf16, tag="sc",
                               name=f"sc{q}", bufs=3)
                pb = psB.tile([128, CQ], f32, tag="psB", name=f"pb{q}")
                for ss in range(CQ // 512):
                    c0 = q * CQ + ss * 512
                    nc.tensor.matmul(pb[:, ss * 512:(ss + 1) * 512],
                                     on_sb, hrow[0:1, c0:c0 + 512],
                                     start=True, stop=True)
                if mode == 'E':
                    for t in range(2):
                        nc.scalar.activation(sc[:, t * CQ:(t + 1) * CQ],
                                             pb[:], AF.Sigmoid,
                                             scale=att[:, t:t + 1])
                else:  # 'C': one ACT psum->bf16 copy, then 4x DVE taylor
                    hb2 = work.tile([128, CQ], bf16, tag="hb2",
                                    name=f"hb2{q}", bufs=2)
                    nc.scalar.activation(hb2[:], pb[:], AF.Copy)
                    for t in range(2):
                        nc.vector.tensor_scalar(
                            sc[:, t * CQ:(t + 1) * CQ], hb2[:],
                            Bcol[:, t:t + 1], Abf[:, t:t + 1],
                            op0=ALU.mult, op1=ALU.add)
                # wide muls: [128, 2, w] views over chunk pieces
                ov = tview(o[:], CQ)
                sv = tview(sc[:], CQ)
                for (rel, jj, co, w_) in xpieces(q * CQ, CQ):
                    xv = tview(xt[jj][:], CHUNKS[jj][1])
                    nc.vector.tensor_tensor(ov[:, :, rel:rel + w_],
                                            xv[:, :, co:co + w_],
                                            sv[:, :, rel:rel + w_],
                                            op=ALU.mult)
                if q == NQ - 1:
                    for hh in range(2):
                        nc.sync.dma_start(
                            out=outd[:, :, q * CQ + hh * 1024:
                                     q * CQ + (hh + 1) * 1024],
                            in_=ov[:, :, hh * 1024:(hh + 1) * 1024])
                else:
                    nc.sync.dma_start(
                        out=outd[:, :, q * CQ:(q + 1) * CQ], in_=o[:])

    nc.compile()
    return nc


_prog_cache = {}
_TRACE = False      # test harness sets True to collect an NTFF profile
_last_res = None    # BassKernelResults of the most recent run


def kernel(x, dct_w, w1, b1, w2, b2, alpha, lap):
    import ml_dtypes

    x = np.asarray(x, dtype=np.float32)
    dct_w = np.asarray(dct_w, dtype=np.float32)
    w1 = np.asarray(w1, dtype=np.float32)
    b1 = np.asarray(b1, dtype=np.float32)
    w2 = np.asarray(w2, dtype=np.float32)
    b2 = np.asarray(b2, dtype=np.float32)
    alpha = float(np.asarray(alpha))
    lap = np.asarray(lap, dtype=np.float64)

    assert np.allclose(lap[0], lap[2]) and np.allclose(lap[:, 0], lap[:, 2])
    a, b = float(lap[0, 0]), float(lap[0, 1])

    m = dct_w.astype(np.float64).mean(axis=0)           # [C]
    S = np.zeros((H, H), dtype=np.float64)
    for h in range(H):
        S[h, _reflect(h - 1, H)] += 1.0
        S[h, _reflect(h + 1, H)] += 1.0
    from math import comb
    G = (alpha * a) * S
    c24 = 1.0 + alpha * float(lap[1, 1])
    P = c24 * np.eye(H) + 4.0 * G
    Q = (alpha * b) * np.eye(H) + G
    mts = [np.linalg.matrix_power(P, 3 - k) @ np.linalg.matrix_power(Q, k)
           * comb(3, k) for k in range(4)]

    bf16 = ml_dtypes.bfloat16

    # bf16 blob [128, 932]: mv 2 | mt 512 | ones 128 | w1t 32 | w2t 256
    cbh = np.zeros((128, 932), dtype=np.float32)
    cbh[:, 0:2] = np.ascontiguousarray(
        m.astype(np.float32).reshape(2, 128).T)
    for k in range(4):
        cbh[:, 2 + 128 * k:2 + 128 * (k + 1)] = mts[k].T
    cbh[0, 514:642] = 1.0
    cbh[:, 642:674] = w1.T.reshape(2, 128, 16).transpose(1, 0, 2).reshape(
        128, 32)
    cbh[0:16, 674:930] = w2.T
    cbh = cbh.astype(bf16)

    # f32 blob [128, 4]: b2c [128,2] | b1 [16] col2
    cfh = np.zeros((128, 4), dtype=np.float32)
    cfh[:, 0:2] = b2.reshape(2, 128).T
    cfh[0:16, 2] = b1

    key = 0
    if key not in _prog_cache:
        _prog_cache[key] = _build_program(key)
    nc = _prog_cache[key]

    xr = x.reshape(B, 2, 128, HW).transpose(0, 2, 1, 3)
    xr = np.ascontiguousarray(xr).astype(bf16)
    consts = {"cb": cbh, "cf": cfh}
    in_maps = [{"xb": xr[i], **consts} for i in range(N_CORES)]

    from concourse.bass_utils import run_bass_kernel_spmd
    res = run_bass_kernel_spmd(nc, in_maps, list(range(N_CORES)),
                               trace=_TRACE)
    global _last_res
    _last_res = res
    out = np.empty((N_CORES, C, H, W), dtype=np.float32)
    for i in range(N_CORES):
        oi = res.results[i]["out"].astype(np.float32)   # [128, 2, HW]
        out[i] = oi.transpose(1, 0, 2).reshape(C, H, W)
    return out


# revision 18
# speedup vs baseline: 1.6858x; 1.1078x over previous
"""Trainium2 Bass kernel for the HCFDA dense-CNN module (bf16 pipeline, v3).

Math (exact reassociations of the reference):
  1. The 256x256 1x1 DCT conv is only consumed through a channel-mean, so
     temp[b,h,w] = sum_c m[c] * x[b,c,h,w]  with  m = dct_w.mean(axis=0).
  2. The 3 reflect-pad diffusion steps collapse (host-side) into
     T3 = sum_k M_k @ T @ (Sw^T)^k  -> 3 shift-adds + 4 matmuls on device.
  3. SE branch: pooled stats -> two tiny FCs -> sigmoid (branches batched,
     weights in bf16).
  out = x * sigmoid(att[c] * heat[hw]),  heat = sigmoid(T3); exact via ACT
  sigmoid(scale=att) or per-channel Taylor sc ~= A[c] + B[c]*heat (max err
  ~2e-4 at the observed heat range).

Measured op rates (HW bench) that drive the design:
  DVE tensor_tensor bf16: 2x (0.55 ns/col), strided 3D views keep 2x ->
    one op covers both channel halves.  tensor_scalar (AP scalars): 4x
    (0.33 ns/col).  tensor_reduce / accum variants / STT: only 1x.
  ACT: 0.9 ns/col any dtype.  gpsimd partition_broadcast [128,2048]:
    ~3.3 us.  PE matmul: ~0.7 ns/col + ~220 ns.
Hence: pooled max/sum via 2x pairwise fold trees into per-chunk partials
(DVE) + ACT copy-accum for a balanced share of the sums; phase B splits
16 output tiles into 5 exact-q (PE heat-broadcast -> ACT sigmoid -> one
wide DVE mul) + 3 taylor-q (Pool heat-broadcast -> 4x DVE tensor_scalar
-> wide DVE mul), hitting the ~23 us HBM write floor with DVE ~20 us.

DMA layout: x host-interleaved to [128, 2, HW]; 6 chunk loads + 2 packed
const loads issued up-front on the Sync queue (nothing dependent ahead);
Tp scatters + hrow ride the Scalar HWDGE queue; outputs on Sync.

Sharding: pure data parallel, one batch element per NeuronCore (B=8).
"""

import numpy as np
from contextlib import ExitStack

B, C, H, W = 8, 256, 128, 128
HW = H * W           # 16384
N_CORES = 8
H0 = 0.4975          # heat-range center for the Taylor-linear sigmoid

CHUNKS = ((0, 1024), (1024, 1024), (2048, 2048), (4096, 4096),
          (8192, 2048), (10240, 2048), (12288, 3584), (15872, 512))

# GEMV psum stages: (hw0, hwlen, psum width)
STAGES = ((0, 8192, 2048), (8192, 4096, 1024), (12288, 4096, 1024))

LASTJ = 7            # tail chunk index (512 wide, handled in finalize)

# phase B: 'E' exact (PE pb + ACT sigmoid + DVE mul), 'P' taylor via
# gpsimd partition_broadcast + 4x DVE tensor_scalar + DVE mul
QMODE = ('E', 'C', 'E', 'C', 'E', 'C', 'E', 'C')
CQ = 2048
NQ = HW // CQ


def _reflect(i, n):
    if i < 0:
        return -i
    if i >= n:
        return 2 * (n - 1) - i
    return i


def _build_program(key_unused):
    from concourse import bass, mybir, tile
    from concourse import bacc

    f32 = mybir.dt.float32
    bf16 = mybir.dt.bfloat16
    AF = mybir.ActivationFunctionType
    ALU = mybir.AluOpType
    AX = mybir.AxisListType

    nc = bacc.Bacc("TRN2", target_bir_lowering=False, debug=False,
                   num_devices=N_CORES)

    xb = nc.dram_tensor("xb", [128, 2, HW], bf16, kind="ExternalInput").ap()
    cb = nc.dram_tensor("cb", [128, 930], bf16, kind="ExternalInput").ap()
    cf = nc.dram_tensor("cf", [128, 20], f32, kind="ExternalInput").ap()
    outd = nc.dram_tensor("out", [128, 2, HW], bf16,
                          kind="ExternalOutput").ap()

    def tview(ap_2d, jsz):
        # [128, 2*jsz] chunk tile -> [128, 2, jsz]
        return ap_2d.rearrange("p (t c) -> p t c", t=2)

    with tile.TileContext(nc) as tc, ExitStack() as ctx:
        const = ctx.enter_context(tc.tile_pool(name="const", bufs=1))
        xpool = ctx.enter_context(tc.tile_pool(name="xp", bufs=1))
        work = ctx.enter_context(tc.tile_pool(name="work", bufs=2))
        stat = ctx.enter_context(tc.tile_pool(name="stat", bufs=1))
        actxA = ExitStack()
        psA = [actxA.enter_context(
            tc.tile_pool(name=f"psA{s}", bufs=1, space="PSUM"))
            for s in range(3)]
        actx = ctx.enter_context(ExitStack())

        # ---- loads: first x chunk, consts, rest of x — all on Sync ----
        xt = {}

        def load_chunk(j):
            joff, jsz = CHUNKS[j]
            xt[j] = xpool.tile([128, 2 * jsz], bf16, tag=f"x{j}",
                               name=f"x{j}")
            nc.sync.dma_start(out=xt[j][:], in_=xb[:, :, joff:joff + jsz])

        load_chunk(0)
        cb_sb = const.tile([128, 930], bf16, tag="cb", name="cb")
        nc.sync.dma_start(out=cb_sb[:], in_=cb)
        cf_sb = const.tile([128, 20], f32, tag="cf", name="cf")
        nc.sync.dma_start(out=cf_sb[:], in_=cf)
        for j in range(1, len(CHUNKS)):
            load_chunk(j)

        # const blob views (bf16): mw0/mw1 = [m_t | W1^T_t] 17 cols each,
        #   then mt 4*128, ones 128 (row0), w2t 256 (rows 0:16)
        mw_sb = [cb_sb[:, 17 * t:17 * (t + 1)] for t in range(2)]
        mt_sb = [cb_sb[:, 34 + 128 * k:34 + 128 * (k + 1)] for k in range(4)]
        on_sb = cb_sb[0:1, 546:674]
        w2_sb = cb_sb[0:16, 674:930]
        # f32 blob: b2c [128,2] | b1 [16,1] col2 | col3 spare | selw [128,16]
        b2c_sb = cf_sb[:, 0:2]
        b1_sb = cf_sb[0:16, 2:3]
        selw_sb = cf_sb[:, 4:20]

        warm = const.tile([1, 2], f32, tag="warm", name="warm")
        nc.scalar.activation(warm[:], cf_sb[0:1, 0:2], AF.Sigmoid)

        wacc = stat.tile([128, 3], f32, tag="wacc", name="wacc")
        Tp = [stat.tile([128, W + 2], bf16, tag=f"Tp{i}", name=f"Tp{i}")
              for i in range(4)]
        heat = stat.tile([128, W], bf16, tag="heat", name="heat")
        hrow = stat.tile([1, HW], bf16, tag="hrow", name="hrow")
        stg = [stat.tile([128, STAGES[s][2]], bf16, tag=f"stg{s}",
                         name=f"stg{s}") for s in range(3)]
        pss = [psA[s].tile([128, STAGES[s][2]], f32, tag=f"ps{s}",
                           name=f"ps{s}") for s in range(3)]

        def stage_of(hw0):
            for s, (s0, slen, sw) in enumerate(STAGES):
                if s0 <= hw0 < s0 + slen:
                    return s, s0, sw
            raise AssertionError(hw0)

        def emit_gemv(j):
            joff, jsz = CHUNKS[j]
            for G0 in range(joff, joff + jsz, 512):
                s, s0, sw = stage_of(G0)
                gl = (G0 - s0) // 512
                ncol = sw // 512
                k, g = gl // ncol, gl % ncol
                dst = pss[s][32 * k:32 * k + 17, 512 * g:512 * g + 512]
                co = G0 - joff
                for t in range(2):
                    nc.tensor.matmul(
                        dst, mw_sb[t],
                        xt[j][:, t * jsz + co:t * jsz + co + 512],
                        start=(t == 0), stop=(t == 1),
                        tile_position=(0, 32 * k))
            for s, (s0, slen, sw) in enumerate(STAGES):
                if s0 + slen == joff + jsz:
                    nc.scalar.activation(stg[s][:], pss[s][:], AF.Copy,
                                         accum_out=wacc[:, s:s + 1])
                    r0 = s0 // 128
                    nr = slen // 128
                    nc.scalar.dma_start(
                        out=Tp[0][r0:r0 + nr, 1:W + 1],
                        in_=stg[s][0:128:32, :])

        # running max accumulator [128, 2, 1024] bf16; per-chunk folds
        # merge in immediately (sums ride the GEMV via the W1 columns).
        nmx = stat.tile([128, 2, 1024], bf16, tag="nmx", name="nmx")

        def fold_tree(x3, jsz, op, tag):
            """fold [128,2,jsz] -> [128,2,1024] AP (2x DVE)."""
            f = work.tile([128, 2, 1024], bf16, tag=tag, name=f"{tag}_x",
                          bufs=2)
            if jsz == 2048:
                nc.vector.tensor_tensor(f[:], x3[:, :, 0:1024],
                                        x3[:, :, 1024:2048], op=op)
            elif jsz == 4096:
                f2 = work.tile([128, 2, 2048], bf16, tag=tag + "w",
                               name=f"{tag}w_x", bufs=1)
                nc.vector.tensor_tensor(f2[:], x3[:, :, 0:2048],
                                        x3[:, :, 2048:4096], op=op)
                nc.vector.tensor_tensor(f[:], f2[:, :, 0:1024],
                                        f2[:, :, 1024:2048], op=op)
            elif jsz == 3584:
                nc.vector.tensor_tensor(f[:], x3[:, :, 0:1024],
                                        x3[:, :, 1024:2048], op=op)
                nc.vector.tensor_tensor(f[:], f[:], x3[:, :, 2048:3072],
                                        op=op)
                nc.vector.tensor_tensor(f[:, :, 0:512], f[:, :, 0:512],
                                        x3[:, :, 3072:3584], op=op)
            else:
                raise AssertionError(jsz)
            return f

        def emit_stats(j):
            joff, jsz = CHUNKS[j]
            if j == 0 or j == LASTJ:
                return
            x3 = tview(xt[j][:], jsz)
            if jsz == 1024:
                f = x3
            else:
                f = fold_tree(x3, jsz, ALU.max, "mf")[:]
            if j == 1:
                nc.vector.tensor_tensor(nmx[:], tview(xt[0][:], 1024),
                                        f, op=ALU.max)
            else:
                nc.vector.tensor_tensor(nmx[:], nmx[:], f, op=ALU.max)

        for j in range(len(CHUNKS)):
            with tc.high_priority():
                emit_gemv(j)
            emit_stats(j)

        # ---- finalize: fold nmx down, then short reduces ----
        ymax = stat.tile([128, 2], f32, tag="ymax", name="ymax")
        m1 = stat.tile([128, 2], f32, tag="m1", name="m1")
        nm2 = stat.tile([128, 2, 256], bf16, tag="nm2", name="nm2")
        nc.vector.tensor_tensor(nm2[:], nmx[:, :, 0:256],
                                nmx[:, :, 256:512], op=ALU.max)
        nc.vector.tensor_tensor(nm2[:], nm2[:], nmx[:, :, 512:768],
                                op=ALU.max)
        nc.vector.tensor_tensor(nm2[:], nm2[:], nmx[:, :, 768:1024],
                                op=ALU.max)
        nc.vector.tensor_reduce(m1[:], nm2[:], axis=AX.X, op=ALU.max)
        x5 = tview(xt[LASTJ][:], 512)
        ymaxb = stat.tile([128, 2], bf16, tag="ymaxb", name="ymaxb")
        for t in range(2):
            j5m = stat.tile([128, 1], f32, tag=f"j5m{t}", name=f"j5m{t}")
            nc.vector.tensor_reduce(j5m[:], x5[:, t:t + 1, :], axis=AX.XY,
                                    op=ALU.max)
            nc.vector.tensor_tensor(ymax[:, t:t + 1], m1[:, t:t + 1],
                                    j5m[:], op=ALU.max)
            nc.vector.tensor_copy(ymaxb[:, t:t + 1], ymax[:, t:t + 1])
        wsum = stat.tile([128, 1], f32, tag="wsum", name="wsum")
        nc.vector.tensor_reduce(wsum[:], wacc[:], axis=AX.X, op=ALU.add)

        # ---- diffusion on Tp + heat (scalar-queue DMAs) ----
        actxA.close()
        psD = actx.enter_context(tc.tile_pool(name="psD", bufs=1,
                                              space="PSUM"))
        psF = actx.enter_context(tc.tile_pool(name="psF", bufs=1,
                                              space="PSUM"))
        with tc.high_priority():
            nc.vector.tensor_copy(Tp[0][:, 0:1], Tp[0][:, 2:3])
            nc.vector.tensor_copy(Tp[0][:, W + 1:W + 2], Tp[0][:, W - 1:W])
            pd3 = psD.tile([128, W], f32, tag="psD", name="psD")
            for k in range(4):
                nc.tensor.matmul(pd3[:], mt_sb[k], Tp[k][:, 1:W + 1],
                                 start=(k == 0), stop=(k == 3))
                if k < 3:
                    nxt = Tp[k + 1]
                    nc.vector.tensor_add(nxt[:, 1:W + 1], Tp[k][:, 0:W],
                                         Tp[k][:, 2:W + 2])
                    nc.vector.tensor_copy(nxt[:, 0:1], nxt[:, 2:3])
                    nc.vector.tensor_copy(nxt[:, W + 1:W + 2],
                                          nxt[:, W - 1:W])
            nc.scalar.activation(heat[:], pd3[:], AF.Sigmoid)
            nc.scalar.dma_start(out=hrow[0:1, 0:HW // 2], in_=heat[0:64, :])
            nc.scalar.dma_start(out=hrow[0:1, HW // 2:HW],
                                in_=heat[64:128, :])

        # ---- SE FC chain: avg branch rides the GEMV (selw @ wsum),
        #      max branch contracts ymax through the same W1 columns ----
        att = stat.tile([128, 2], f32, tag="att", name="att")
        ph = psF.tile([16, 2], f32, tag="psF", name="ph")
        nc.tensor.matmul(ph[:, 0:1], selw_sb, wsum[:],
                         start=True, stop=True)
        nc.tensor.matmul(ph[:, 1:2], mw_sb[0][:, 1:17], ymaxb[:, 0:1],
                         start=True, stop=False)
        nc.tensor.matmul(ph[:, 1:2], mw_sb[1][:, 1:17], ymaxb[:, 1:2],
                         start=False, stop=True)
        hb = stat.tile([16, 2], bf16, tag="hb", name="hb")
        nc.scalar.activation(hb[:], ph[:], AF.Relu, bias=b1_sb)
        for t in range(2):
            pa = psF.tile([128, 2], f32, tag="psFa", name=f"pa{t}")
            nc.tensor.matmul(pa[:], w2_sb[:, 128 * t:128 * (t + 1)], hb[:],
                             start=True, stop=True)
            sg = stat.tile([128, 2], f32, tag=f"sg{t}", name=f"sg{t}")
            nc.scalar.activation(sg[:], pa[:], AF.Sigmoid,
                                 bias=b2c_sb[:, t:t + 1])
            nc.vector.tensor_add(att[:, t:t + 1], sg[:, 0:1], sg[:, 1:2])

        # Taylor coeffs (column form only): sc ~= A + B*heat
        uat = stat.tile([128, 2], f32, tag="uat", name="uat")
        nc.vector.tensor_scalar_mul(uat[:], att[:], H0)
        sat = stat.tile([128, 2], f32, tag="sat", name="sat")
        nc.scalar.activation(sat[:], uat[:], AF.Sigmoid)
        spt = stat.tile([128, 2], f32, tag="spt", name="spt")
        nc.vector.tensor_mul(spt[:], sat[:], sat[:])
        nc.vector.tensor_sub(spt[:], sat[:], spt[:])       # s*(1-s)
        Abf = stat.tile([128, 2], f32, tag="Abf", name="Abf")
        nc.vector.tensor_mul(Abf[:], uat[:], spt[:])
        nc.vector.tensor_sub(Abf[:], sat[:], Abf[:])
        Bcol = stat.tile([128, 2], f32, tag="Bcol", name="Bcol")
        nc.vector.tensor_mul(Bcol[:], att[:], spt[:])

        # ---- Phase B ----
        actx.close()

        def xpieces(hw0, width):
            out = []
            pos = hw0
            while pos < hw0 + width:
                for jj, (joff, jsz) in enumerate(CHUNKS):
                    if joff <= pos < joff + jsz:
                        w_ = min(hw0 + width, joff + jsz) - pos
                        out.append((pos - hw0, jj, pos - joff, w_))
                        pos += w_
                        break
                else:
                    raise AssertionError(pos)
            return out

        with tc.tile_pool(name="psB", bufs=2, space="PSUM") as psB:
            for q in range(NQ):
                mode = QMODE[q]
                o = work.tile([128, 2 * CQ], bf16, tag="o", name=f"o{q}",
                              bufs=3)
                sc = work.tile([128, 2 * CQ], bf16, tag="sc",
                               name=f"sc{q}", bufs=3)
                pb = psB.tile([128, CQ], f32, tag="psB", name=f"pb{q}")
                for ss in range(CQ // 512):
                    c0 = q * CQ + ss * 512
                    nc.tensor.matmul(pb[:, ss * 512:(ss + 1) * 512],
                                     on_sb, hrow[0:1, c0:c0 + 512],
                                     start=True, stop=True)
                if mode == 'E':
                    for t in range(2):
                        nc.scalar.activation(sc[:, t * CQ:(t + 1) * CQ],
                                             pb[:], AF.Sigmoid,
                                             scale=att[:, t:t + 1])
                else:  # 'C': one ACT psum->bf16 copy, then 4x DVE taylor
                    hb2 = work.tile([128, CQ], bf16, tag="hb2",
                                    name=f"hb2{q}", bufs=2)
                    nc.scalar.activation(hb2[:], pb[:], AF.Copy)
                    for t in range(2):
                        nc.vector.tensor_scalar(
                            sc[:, t * CQ:(t + 1) * CQ], hb2[:],
                            Bcol[:, t:t + 1], Abf[:, t:t + 1],
                            op0=ALU.mult, op1=ALU.add)
                # wide muls: [128, 2, w] views over chunk pieces
                ov = tview(o[:], CQ)
                sv = tview(sc[:], CQ)
                for (rel, jj, co, w_) in xpieces(q * CQ, CQ):
                    xv = tview(xt[jj][:], CHUNKS[jj][1])
                    nc.vector.tensor_tensor(ov[:, :, rel:rel + w_],
                                            xv[:, :, co:co + w_],
                                            sv[:, :, rel:rel + w_],
                                            op=ALU.mult)
                if q == NQ - 1:
                    for hh in range(2):
                        nc.sync.dma_start(
                            out=outd[:, :, q * CQ + hh * 1024:
                                     q * CQ + (hh + 1) * 1024],
                            in_=ov[:, :, hh * 1024:(hh + 1) * 1024])
                else:
                    nc.sync.dma_start(
                        out=outd[:, :, q * CQ:(q + 1) * CQ], in_=o[:])

    nc.compile()
    return nc


_prog_cache = {}
_TRACE = False      # test harness sets True to collect an NTFF profile
_last_res = None    # BassKernelResults of the most recent run


def kernel(x, dct_w, w1, b1, w2, b2, alpha, lap):
    import ml_dtypes

    x = np.asarray(x, dtype=np.float32)
    dct_w = np.asarray(dct_w, dtype=np.float32)
    w1 = np.asarray(w1, dtype=np.float32)
    b1 = np.asarray(b1, dtype=np.float32)
    w2 = np.asarray(w2, dtype=np.float32)
    b2 = np.asarray(b2, dtype=np.float32)
    alpha = float(np.asarray(alpha))
    lap = np.asarray(lap, dtype=np.float64)

    assert np.allclose(lap[0], lap[2]) and np.allclose(lap[:, 0], lap[:, 2])
    a, b = float(lap[0, 0]), float(lap[0, 1])

    m = dct_w.astype(np.float64).mean(axis=0)           # [C]
    S = np.zeros((H, H), dtype=np.float64)
    for h in range(H):
        S[h, _reflect(h - 1, H)] += 1.0
        S[h, _reflect(h + 1, H)] += 1.0
    from math import comb
    G = (alpha * a) * S
    c24 = 1.0 + alpha * float(lap[1, 1])
    P = c24 * np.eye(H) + 4.0 * G
    Q = (alpha * b) * np.eye(H) + G
    mts = [np.linalg.matrix_power(P, 3 - k) @ np.linalg.matrix_power(Q, k)
           * comb(3, k) for k in range(4)]

    bf16 = ml_dtypes.bfloat16

    # bf16 blob [128, 930]: mw0 17 | mw1 17 | mt 512 | ones 128 | w2t 256
    cbh = np.zeros((128, 930), dtype=np.float32)
    mv = np.ascontiguousarray(m.astype(np.float32).reshape(2, 128).T)
    w1T = w1.T      # [256, 16]
    for t in range(2):
        cbh[:, 17 * t] = mv[:, t]
        cbh[:, 17 * t + 1:17 * (t + 1)] = w1T[128 * t:128 * (t + 1), :]
    for k in range(4):
        cbh[:, 34 + 128 * k:34 + 128 * (k + 1)] = mts[k].T
    cbh[0, 546:674] = 1.0
    cbh[0:16, 674:930] = w2.T
    cbh = cbh.astype(bf16)

    # f32 blob [128, 20]: b2c [128,2] | b1 col2 | spare | selw [128,16]
    cfh = np.zeros((128, 20), dtype=np.float32)
    cfh[:, 0:2] = b2.reshape(2, 128).T
    cfh[0:16, 2] = b1
    for k in range(4):
        for j in range(16):
            cfh[32 * k + 1 + j, 4 + j] = 1.0 / HW

    key = 0
    if key not in _prog_cache:
        _prog_cache[key] = _build_program(key)
    nc = _prog_cache[key]

    xr = x.reshape(B, 2, 128, HW).transpose(0, 2, 1, 3)
    xr = np.ascontiguousarray(xr).astype(bf16)
    consts = {"cb": cbh, "cf": cfh}
    in_maps = [{"xb": xr[i], **consts} for i in range(N_CORES)]

    from concourse.bass_utils import run_bass_kernel_spmd
    res = run_bass_kernel_spmd(nc, in_maps, list(range(N_CORES)),
                               trace=_TRACE)
    global _last_res
    _last_res = res
    out = np.empty((N_CORES, C, H, W), dtype=np.float32)
    for i in range(N_CORES):
        oi = res.results[i]["out"].astype(np.float32)   # [128, 2, HW]
        out[i] = oi.transpose(1, 0, 2).reshape(C, H, W)
    return out
